# revision 10
# baseline (speedup 1.0000x reference)
"""DiffPool forward (GCN embed/assign + pooled X/A) on 8 trn2 NeuronCores.

Sharding: 1D node partition. Core c owns rows R_c = [c*ROWS, (c+1)*ROWS).
Host feeds each core A_T_hat_c = (A[R_c,:] + I[R_c,:]).T  -> [N, ROWS], so the
contraction index j sits on the SBUF partition axis for both big products and
no transposes of A are ever done on device.

Device program (single SPMD NEFF, collectives across 8 cores):
  P1: deg_loc = column-sums of A_T_hat strips (DVE adds + partition tree)
      AllGather(deg) -> d = 1/sqrt(deg)
  P2: AHXT[f,i]   = sum_j (d*X)[j,f] * A_T_hat[j,i]   (PSUM accum over 64 j-blocks)
      z   = d_i * (AHXT.T @ We.T) + be
      s   = softmax(d_i * (AHXT.T @ Wa.T) + ba)       (row softmax, free axis)
      AllGather(s) -> s_full
  P3: AhatS_T[m,i] = sum_j s_full[j,m] * A_T_hat[j,i] (same strip pattern)
      As = AhatS_T.T - s_loc                          (undo baked +I)
      Xn_part = s_loc.T @ z ; An_part = s_loc.T @ As  (partials over local rows)
Host: X_next = sum_c Xn_part, A_next = sum_c An_part, s_l = concat_c(s_part).
"""

import numpy as np

import concourse.bass as bass
import concourse.mybir as mybir
import concourse.tile as tile
from concourse import bacc
from concourse.bass_utils import run_bass_kernel_spmd
from concourse.masks import make_identity

F = 128  # F_IN == N_META == N_EMBED == 128
f32 = mybir.dt.float32
N_FULL = 8192
CORES_FULL = 8


def build_program(n=N_FULL, n_cores=CORES_FULL):
    """Build the SPMD single-core program (same NEFF on all cores)."""
    rows = n // n_cores      # local node rows per core
    nb = n // 128            # j blocks (global)
    rb = rows // 128         # local row blocks
    ch = min(512, rows)      # matmul moving-dim chunk
    nch = rows // ch         # chunks per strip
    assert nb % 2 == 0 and rows % 128 == 0

    nc = bacc.Bacc(
        "TRN2",
        target_bir_lowering=False,
        debug=False,
        enable_asserts=False,
        num_devices=n_cores,
    )

    at = nc.dram_tensor("at_hat", [n, rows], f32, kind="ExternalInput")
    x_in = nc.dram_tensor("x_in", [n, F], f32, kind="ExternalInput")
    wet = nc.dram_tensor("we_t", [F, F], f32, kind="ExternalInput")
    wat = nc.dram_tensor("wa_t", [F, F], f32, kind="ExternalInput")
    beb = nc.dram_tensor("be_b", [128, F], f32, kind="ExternalInput")
    bab = nc.dram_tensor("ba_b", [128, F], f32, kind="ExternalInput")

    s_out = nc.dram_tensor("s_part", [rows, F], f32, kind="ExternalOutput")
    xn_out = nc.dram_tensor("xn_part", [F, F], f32, kind="ExternalOutput")
    an_out = nc.dram_tensor("an_part", [F, F], f32, kind="ExternalOutput")

    cc_deg_in = nc.dram_tensor("cc_deg_in", [rows], f32, kind="Internal")
    cc_deg_out = nc.dram_tensor(
        "cc_deg_out", [n], f32, kind="Internal", addr_space="Shared"
    )
    cc_s_in = nc.dram_tensor("cc_s_in", [rows, F], f32, kind="Internal")
    cc_s_out = nc.dram_tensor(
        "cc_s_out", [n, F], f32, kind="Internal", addr_space="Shared"
    )
    rgroups = [list(range(n_cores))]

    with tile.TileContext(nc) as tc:
        with (
            tc.tile_pool(name="strips", bufs=8) as strips,
            tc.tile_pool(name="persist", bufs=1) as persist,
            tc.tile_pool(name="sf", bufs=4) as sfp,
            tc.tile_pool(name="pacc", bufs=1, space="PSUM") as pacc,
            tc.tile_pool(name="pmm", bufs=2, space="PSUM") as pmm,
            tc.tile_pool(name="pfin", bufs=1, space="PSUM") as pfin,
        ):
            # ---- persistent sbuf tensors ----
            x_sb = persist.tile([128, nb, F], f32)      # X, later d*X (j on partitions)
            acc = persist.tile([128, rows], f32)        # P1 accumulator
            d_sb = persist.tile([128, nb], f32)         # 1/sqrt(deg) full, [p, blk]
            d_rc = persist.tile([128, rb], f32)         # same for local rows
            wet_sb = persist.tile([F, F], f32)
            wat_sb = persist.tile([F, F], f32)
            beb_sb = persist.tile([128, F], f32)
            bab_sb = persist.tile([128, F], f32)
            ident = persist.tile([128, 128], f32)
            axt_sb = persist.tile([128, rows], f32)     # AHXT
            z_sb = persist.tile([128, rb, F], f32)
            sl_sb = persist.tile([128, rb, F], f32)     # logits -> s
            as_sb = persist.tile([128, rb, F], f32)     # A @ s (local rows)
            ast_sb = persist.tile([128, rows], f32)     # AhatS_T
            ones_sb = persist.tile([128, 1], f32)
            degline = persist.tile([1, rows], f32)
            mx = persist.tile([128, rb, 1], f32)
            ssum = persist.tile([128, rb, 1], f32)
            rsum = persist.tile([128, rb, 1], f32)
            xn_sb = persist.tile([F, F], f32)
            an_sb = persist.tile([F, F], f32)

            # ---- constant loads / setup ----
            nc.sync.dma_start(x_sb[:], x_in.ap().rearrange("(b p) f -> p b f", p=128))
            nc.sync.dma_start(wet_sb[:], wet.ap())
            nc.sync.dma_start(wat_sb[:], wat.ap())
            nc.sync.dma_start(beb_sb[:], beb.ap())
            nc.sync.dma_start(bab_sb[:], bab.ap())
            make_identity(nc, ident[:])
            nc.vector.memset(acc[:], 0.0)
            nc.vector.memset(ones_sb[:], 1.0)

            # ---- P1: degree = column sums of A_T_hat ----
            for it in range(nb // 2):
                buf = strips.tile([128, 2, rows], f32, tag="abuf")
                nc.sync.dma_start(
                    buf[:],
                    at.ap()[it * 256 : (it + 1) * 256, :].rearrange(
                        "(t p) r -> p t r", p=128
                    ),
                )
                nc.vector.tensor_add(acc[:], acc[:], buf[:, 0, :])
                nc.vector.tensor_add(acc[:], acc[:], buf[:, 1, :])
            # partition reduction via ones-vector matmul: degline = sum_p acc[p,:]
            for c in range(nch):
                deg_ps = pmm.tile([1, ch], f32, tag="degps")
                nc.tensor.matmul(
                    deg_ps[:], ones_sb[:], acc[:, c * ch : (c + 1) * ch]
                )
                nc.vector.tensor_copy(degline[:, c * ch : (c + 1) * ch], deg_ps[:])

            # AllGather degrees; local d read back from the DRAM bounce so no
            # core-dependent indexing is needed anywhere in the program
            nc.gpsimd.dma_start(
                cc_deg_in.ap().rearrange("(a b) -> a b", a=1), degline[:]
            )
            nc.sync.dma_start(
                d_rc[:], cc_deg_in.ap().rearrange("(b p) -> p b", p=128)
            )
            nc.scalar.sqrt(d_rc[:], d_rc[:])
            nc.vector.reciprocal(d_rc[:], d_rc[:])
            nc.gpsimd.collective_compute(
                "AllGather",
                mybir.AluOpType.bypass,
                replica_groups=rgroups,
                ins=[cc_deg_in.ap()],
                outs=[cc_deg_out.ap()],
            )
            nc.sync.dma_start(
                d_sb[:], cc_deg_out.ap().rearrange("(b p) -> p b", p=128)
            )
            nc.scalar.sqrt(d_sb[:], d_sb[:])
            nc.vector.reciprocal(d_sb[:], d_sb[:])

            # X <- d * X (rows j scaled by d[j])
            for b in range(nb):
                nc.vector.tensor_scalar_mul(
                    x_sb[:, b, :], x_sb[:, b, :], d_sb[:, b : b + 1]
                )

            # ---- P2: AHXT accumulation over j blocks ----
            axt_ps = pacc.tile([128, rows], f32, tag="bigacc")
            for it in range(nb // 2):
                buf = strips.tile([128, 2, rows], f32, tag="abuf")
                nc.sync.dma_start(
                    buf[:],
                    at.ap()[it * 256 : (it + 1) * 256, :].rearrange(
                        "(t p) r -> p t r", p=128
                    ),
                )
                for t in range(2):
                    jb = 2 * it + t
                    for c in range(nch):
                        nc.tensor.matmul(
                            axt_ps[:, c * ch : (c + 1) * ch],
                            x_sb[:, jb, :],
                            buf[:, t, c * ch : (c + 1) * ch],
                            start=(jb == 0),
                            stop=(jb == nb - 1),
                        )
            for c in range(nch):
                nc.vector.tensor_copy(
                    axt_sb[:, c * ch : (c + 1) * ch], axt_ps[:, c * ch : (c + 1) * ch]
                )

            # ---- z, softmax(s) per local row block ----
            for ib in range(rb):
                z_ps = pmm.tile([128, F], f32, tag="zps")
                nc.tensor.matmul(
                    z_ps[:], axt_sb[:, ib * 128 : (ib + 1) * 128], wet_sb[:]
                )
                nc.vector.scalar_tensor_tensor(
                    z_sb[:, ib, :],
                    z_ps[:],
                    d_rc[:, ib : ib + 1],
                    beb_sb[:],
                    op0=mybir.AluOpType.mult,
                    op1=mybir.AluOpType.add,
                )
                sl_ps = pmm.tile([128, F], f32, tag="zps")
                nc.tensor.matmul(
                    sl_ps[:], axt_sb[:, ib * 128 : (ib + 1) * 128], wat_sb[:]
                )
                nc.vector.scalar_tensor_tensor(
                    sl_sb[:, ib, :],
                    sl_ps[:],
                    d_rc[:, ib : ib + 1],
                    bab_sb[:],
                    op0=mybir.AluOpType.mult,
                    op1=mybir.AluOpType.add,
                )
            nc.vector.reduce_max(
                mx[:], sl_sb[:], axis=mybir.AxisListType.X, negate=True
            )
            for ib in range(rb):
                nc.scalar.activation(
                    sl_sb[:, ib, :],
                    sl_sb[:, ib, :],
                    mybir.ActivationFunctionType.Exp,
                    bias=mx[:, ib, :],
                    accum_out=ssum[:, ib, :],
                )
            nc.vector.reciprocal(rsum[:], ssum[:])
            for ib in range(rb):
                nc.vector.tensor_scalar_mul(
                    sl_sb[:, ib, :], sl_sb[:, ib, :], rsum[:, ib, :]
                )

            # s shard out + AllGather
            nc.sync.dma_start(
                s_out.ap().rearrange("(b p) f -> p b f", p=128), sl_sb[:]
            )
            nc.gpsimd.dma_start(
                cc_s_in.ap().rearrange("(b p) f -> p b f", p=128), sl_sb[:]
            )
            nc.gpsimd.collective_compute(
                "AllGather",
                mybir.AluOpType.bypass,
                replica_groups=rgroups,
                ins=[cc_s_in.ap()],
                outs=[cc_s_out.ap()],
            )

            # ---- P3: AhatS_T accumulation over j blocks ----
            ast_ps = pacc.tile([128, rows], f32, tag="bigacc")
            for it in range(nb // 2):
                buf = strips.tile([128, 2, rows], f32, tag="abuf")
                nc.sync.dma_start(
                    buf[:],
                    at.ap()[it * 256 : (it + 1) * 256, :].rearrange(
                        "(t p) r -> p t r", p=128
                    ),
                )
                for t in range(2):
                    jb = 2 * it + t
                    sf = sfp.tile([128, F], f32, tag="sfull")
                    nc.sync.dma_start(
                        sf[:], cc_s_out.ap()[jb * 128 : (jb + 1) * 128, :]
                    )
                    for c in range(nch):
                        nc.tensor.matmul(
                            ast_ps[:, c * ch : (c + 1) * ch],
                            sf[:],
                            buf[:, t, c * ch : (c + 1) * ch],
                            start=(jb == 0),
                            stop=(jb == nb - 1),
                        )
            for c in range(nch):
                nc.vector.tensor_copy(
                    ast_sb[:, c * ch : (c + 1) * ch], ast_ps[:, c * ch : (c + 1) * ch]
                )

            # As = AhatS_T.T - s_loc ; then final contractions
            xn_ps = pfin.tile([F, F], f32, tag="xnps")
            an_ps = pfin.tile([F, F], f32, tag="anps")
            for ib in range(rb):
                tr_ps = pmm.tile([128, 128], f32, tag="zps")
                nc.tensor.transpose(
                    tr_ps[:], ast_sb[:, ib * 128 : (ib + 1) * 128], ident[:]
                )
                nc.vector.tensor_sub(as_sb[:, ib, :], tr_ps[:], sl_sb[:, ib, :])
            for ib in range(rb):
                nc.tensor.matmul(
                    xn_ps[:],
                    sl_sb[:, ib, :],
                    z_sb[:, ib, :],
                    start=(ib == 0),
                    stop=(ib == rb - 1),
                )
                nc.tensor.matmul(
                    an_ps[:],
                    sl_sb[:, ib, :],
                    as_sb[:, ib, :],
                    start=(ib == 0),
                    stop=(ib == rb - 1),
                )
            nc.vector.tensor_copy(xn_sb[:], xn_ps[:])
            nc.vector.tensor_copy(an_sb[:], an_ps[:])
            nc.sync.dma_start(xn_out.ap(), xn_sb[:])
            nc.sync.dma_start(an_out.ap(), an_sb[:])

    nc.compile()
    return nc


def make_in_maps(X, A, W_embed, b_embed, W_assign, b_assign, n_cores=CORES_FULL):
    """Host-side sharding / layout prep. Returns one input dict per core."""
    X = np.ascontiguousarray(np.asarray(X, dtype=np.float32))
    A = np.asarray(A, dtype=np.float32)
    n = A.shape[0]
    rows = n // n_cores
    we_t = np.ascontiguousarray(np.asarray(W_embed, np.float32).T)
    wa_t = np.ascontiguousarray(np.asarray(W_assign, np.float32).T)
    be_b = np.ascontiguousarray(
        np.broadcast_to(np.asarray(b_embed, np.float32)[None, :], (128, F))
    )
    ba_b = np.ascontiguousarray(
        np.broadcast_to(np.asarray(b_assign, np.float32)[None, :], (128, F))
    )
    in_maps = []
    for c in range(n_cores):
        at = np.ascontiguousarray(A[c * rows : (c + 1) * rows, :].T)
        # bake A_hat = A + I into the shard
        at[np.arange(c * rows, (c + 1) * rows), np.arange(rows)] += 1.0
        in_maps.append(
            {
                "at_hat": at,
                "x_in": X,
                "we_t": we_t,
                "wa_t": wa_t,
                "be_b": be_b,
                "ba_b": ba_b,
            }
        )
    return in_maps


_CACHE = {}


def _get_program(n, n_cores):
    key = (n, n_cores)
    if key not in _CACHE:
        _CACHE[key] = build_program(n, n_cores)
    return _CACHE[key]


def run_on_hw(inputs, n_cores=CORES_FULL, trace=False):
    """Compile (cached), run on hardware, return (outputs_tuple, BassKernelResults)."""
    from concourse.bass_interp import get_hw_module

    n = inputs["A"].shape[0]
    nc = _get_program(n, n_cores)
    in_maps = make_in_maps(n_cores=n_cores, **inputs)
    old_m = nc.m
    nc.m = get_hw_module(nc.m)
    try:
        res = run_bass_kernel_spmd(
            nc, in_maps, core_ids=list(range(n_cores)), trace=trace
        )
    finally:
        nc.m = old_m
    outs = res.results
    x_next = np.sum([r["xn_part"] for r in outs], axis=0, dtype=np.float32)
    a_next = np.sum([r["an_part"] for r in outs], axis=0, dtype=np.float32)
    s_l = np.concatenate([r["s_part"] for r in outs], axis=0)
    return (x_next, a_next, s_l), res


def kernel(X, A, W_embed, b_embed, W_assign, b_assign):
    out, _ = run_on_hw(
        dict(
            X=X,
            A=A,
            W_embed=W_embed,
            b_embed=b_embed,
            W_assign=W_assign,
            b_assign=b_assign,
        )
    )
    return out


# revision 22
# speedup vs baseline: 1.1269x; 1.1269x over previous
"""DiffPool forward (GCN embed/assign + pooled X/A) on 8 trn2 NeuronCores.

Sharding: 1D node partition. Core c owns rows R_c = [c*ROWS, (c+1)*ROWS).
Host feeds each core A_T_hat_c = (A[R_c,:] + I[R_c,:]).T  -> [N, ROWS], so the
contraction index j sits on the SBUF partition axis for both big products and
no transposes of A are ever done on device.

Device program (single SPMD NEFF, collectives across 8 cores):
  P1: deg_loc = column-sums of A_T_hat strips (DVE adds + partition tree)
      AllGather(deg) -> d = 1/sqrt(deg)
  P2: AHXT[f,i]   = sum_j (d*X)[j,f] * A_T_hat[j,i]   (PSUM accum over 64 j-blocks)
      z   = d_i * (AHXT.T @ We.T) + be
      s   = softmax(d_i * (AHXT.T @ Wa.T) + ba)       (row softmax, free axis)
      AllGather(s) -> s_full
  P3: AhatS_T[m,i] = sum_j s_full[j,m] * A_T_hat[j,i] (same strip pattern)
      As = AhatS_T.T - s_loc                          (undo baked +I)
      Xn_part = s_loc.T @ z ; An_part = s_loc.T @ As  (partials over local rows)
Host: X_next = sum_c Xn_part, A_next = sum_c An_part, s_l = concat_c(s_part).
"""

import numpy as np

import concourse.bass as bass
import concourse.mybir as mybir
import concourse.tile as tile
from concourse import bacc
from concourse.bass_utils import run_bass_kernel_spmd
from concourse.masks import make_identity

F = 128  # F_IN == N_META == N_EMBED == 128
f32 = mybir.dt.float32
N_FULL = 8192
CORES_FULL = 8


def build_program(n=N_FULL, n_cores=CORES_FULL, fast32=True):
    """Build the SPMD single-core program (same NEFF on all cores)."""
    rows = n // n_cores      # local node rows per core
    nb = n // 128            # j blocks (global)
    rb = rows // 128         # local row blocks
    ch = min(512, rows)      # matmul moving-dim chunk
    nch = rows // ch         # chunks per strip
    assert nb % 2 == 0 and rows % 128 == 0
    mmdt = mybir.dt.float32r if fast32 else f32

    def mm(ap):
        return ap.bitcast(mmdt)

    nc = bacc.Bacc(
        "TRN2",
        target_bir_lowering=False,
        debug=False,
        enable_asserts=False,
        num_devices=n_cores,
    )

    at = nc.dram_tensor("at_hat", [n, rows], f32, kind="ExternalInput")
    anat = nc.dram_tensor("a_nat", [rows, n], f32, kind="ExternalInput")
    x_in = nc.dram_tensor("x_in", [n, F], f32, kind="ExternalInput")
    wet = nc.dram_tensor("we_t", [F, F], f32, kind="ExternalInput")
    wat = nc.dram_tensor("wa_t", [F, F], f32, kind="ExternalInput")
    beb = nc.dram_tensor("be_b", [128, F], f32, kind="ExternalInput")
    bab = nc.dram_tensor("ba_b", [128, F], f32, kind="ExternalInput")

    s_out = nc.dram_tensor("s_part", [rows, F], f32, kind="ExternalOutput")
    xn_out = nc.dram_tensor("xn_part", [F, F], f32, kind="ExternalOutput")
    an_out = nc.dram_tensor("an_part", [F, F], f32, kind="ExternalOutput")

    cc_deg_in = nc.dram_tensor("cc_deg_in", [rows], f32, kind="Internal")
    cc_deg_out = nc.dram_tensor(
        "cc_deg_out", [n], f32, kind="Internal", addr_space="Shared"
    )
    cc_s_in = nc.dram_tensor("cc_s_in", [rows, F], f32, kind="Internal")
    cc_s_out = nc.dram_tensor(
        "cc_s_out", [n, F], f32, kind="Internal", addr_space="Shared"
    )
    rgroups = [list(range(n_cores))]

    cw = min(4096, n)        # P1 natural-tile column width
    nh = n // cw             # column halves per local row block

    with tile.TileContext(nc) as tc:
        with (
            tc.tile_pool(name="strips", bufs=8) as strips,
            tc.tile_pool(name="natb", bufs=3) as natb,
            tc.tile_pool(name="persist", bufs=1) as persist,
            tc.tile_pool(name="sf", bufs=4) as sfp,
            tc.tile_pool(name="pacc", bufs=1, space="PSUM") as pacc,
            tc.tile_pool(name="pmm", bufs=2, space="PSUM") as pmm,
            tc.tile_pool(name="pfin", bufs=1, space="PSUM") as pfin,
        ):
            # ---- persistent sbuf tensors ----
            x_sb = persist.tile([128, nb, F], f32)      # X, later d*X (j on partitions)
            degparts = persist.tile([128, rb, nh], f32)
            deg2d = persist.tile([128, rb], f32)
            d_sb = persist.tile([128, nb], f32)         # 1/sqrt(deg) full, [p, blk]
            d_rc = persist.tile([128, rb], f32)         # same for local rows
            wet_sb = persist.tile([F, F], f32)
            wat_sb = persist.tile([F, F], f32)
            beb_sb = persist.tile([128, F], f32)
            bab_sb = persist.tile([128, F], f32)
            ident = persist.tile([128, 128], f32)
            axt_sb = persist.tile([128, rows], f32)     # AHXT
            z_sb = persist.tile([128, rb, F], f32)
            sl_sb = persist.tile([128, rb, F], f32)     # logits -> s
            as_sb = persist.tile([128, rb, F], f32)     # A @ s (local rows)
            ast_sb = persist.tile([128, rows], f32)     # AhatS_T
            mx = persist.tile([128, rb, 1], f32)
            ssum = persist.tile([128, rb, 1], f32)
            rsum = persist.tile([128, rb, 1], f32)
            xn_sb = persist.tile([F, F], f32)
            an_sb = persist.tile([F, F], f32)

            # ---- constant loads / setup ----
            nc.sync.dma_start(
                mm(x_sb[:]), x_in.ap().rearrange("(b p) f -> p b f", p=128).bitcast(mmdt)
            )
            nc.sync.dma_start(wet_sb[:], wet.ap())
            nc.sync.dma_start(wat_sb[:], wat.ap())
            nc.sync.dma_start(beb_sb[:], beb.ap())
            nc.sync.dma_start(bab_sb[:], bab.ap())
            make_identity(nc, ident[:])

            # ---- P1: degrees = row sums of the natural-layout shard ----
            for rbi in range(rb):
                for h in range(nh):
                    t = natb.tile([128, cw], f32, tag="nbuf")
                    nc.sync.dma_start(
                        t[:],
                        anat.ap()[rbi * 128 : (rbi + 1) * 128, h * cw : (h + 1) * cw],
                    )
                    nc.vector.reduce_sum(
                        degparts[:, rbi, h : h + 1], t[:], axis=mybir.AxisListType.X
                    )
            # combine halves and add the A_hat self-loop (+1)
            if nh == 2:
                nc.vector.scalar_tensor_tensor(
                    deg2d[:],
                    degparts[:, :, 0],
                    1.0,
                    degparts[:, :, 1],
                    op0=mybir.AluOpType.add,
                    op1=mybir.AluOpType.add,
                )
            else:
                assert nh == 1
                nc.vector.tensor_scalar_add(deg2d[:], degparts[:, :, 0], 1.0)

            # AllGather degrees; local d read back from the DRAM bounce so no
            # core-dependent indexing is needed anywhere in the program
            nc.gpsimd.dma_start(
                cc_deg_in.ap().rearrange("(b p) -> p b", p=128), deg2d[:]
            )
            nc.sync.dma_start(
                d_rc[:], cc_deg_in.ap().rearrange("(b p) -> p b", p=128)
            )
            nc.scalar.sqrt(d_rc[:], d_rc[:])
            nc.vector.reciprocal(d_rc[:], d_rc[:])
            nc.gpsimd.collective_compute(
                "AllGather",
                mybir.AluOpType.bypass,
                replica_groups=rgroups,
                ins=[cc_deg_in.ap()],
                outs=[cc_deg_out.ap()],
            )
            nc.sync.dma_start(
                d_sb[:], cc_deg_out.ap().rearrange("(b p) -> p b", p=128)
            )
            nc.scalar.sqrt(d_sb[:], d_sb[:])
            nc.vector.reciprocal(d_sb[:], d_sb[:])

            # X <- d * X (rows j scaled by d[j]); written as f32r for the PE
            for b in range(nb):
                nc.vector.tensor_scalar_mul(
                    mm(x_sb[:, b, :]), x_sb[:, b, :], d_sb[:, b : b + 1]
                )

            # ---- P2: AHXT accumulation over j blocks ----
            axt_ps = pacc.tile([128, rows], f32, tag="bigacc")
            for it in range(nb // 2):
                buf = strips.tile([128, 2, rows], f32, tag="abuf")
                nc.sync.dma_start(
                    mm(buf[:]),
                    at.ap()[it * 256 : (it + 1) * 256, :]
                    .rearrange("(t p) r -> p t r", p=128)
                    .bitcast(mmdt),
                )
                for t in range(2):
                    jb = 2 * it + t
                    for c in range(nch):
                        nc.tensor.matmul(
                            axt_ps[:, c * ch : (c + 1) * ch],
                            mm(x_sb[:, jb, :]),
                            mm(buf[:, t, c * ch : (c + 1) * ch]),
                            start=(jb == 0),
                            stop=(jb == nb - 1),
                        )
            for c in range(nch):
                nc.vector.tensor_copy(
                    axt_sb[:, c * ch : (c + 1) * ch], axt_ps[:, c * ch : (c + 1) * ch]
                )

            # ---- z, softmax(s) per local row block ----
            for ib in range(rb):
                z_ps = pmm.tile([128, F], f32, tag="zps")
                nc.tensor.matmul(
                    z_ps[:], axt_sb[:, ib * 128 : (ib + 1) * 128], wet_sb[:]
                )
                nc.vector.scalar_tensor_tensor(
                    z_sb[:, ib, :],
                    z_ps[:],
                    d_rc[:, ib : ib + 1],
                    beb_sb[:],
                    op0=mybir.AluOpType.mult,
                    op1=mybir.AluOpType.add,
                )
                sl_ps = pmm.tile([128, F], f32, tag="zps")
                nc.tensor.matmul(
                    sl_ps[:], axt_sb[:, ib * 128 : (ib + 1) * 128], wat_sb[:]
                )
                nc.vector.scalar_tensor_tensor(
                    sl_sb[:, ib, :],
                    sl_ps[:],
                    d_rc[:, ib : ib + 1],
                    bab_sb[:],
                    op0=mybir.AluOpType.mult,
                    op1=mybir.AluOpType.add,
                )
            nc.vector.reduce_max(
                mx[:], sl_sb[:], axis=mybir.AxisListType.X, negate=True
            )
            for ib in range(rb):
                nc.scalar.activation(
                    sl_sb[:, ib, :],
                    sl_sb[:, ib, :],
                    mybir.ActivationFunctionType.Exp,
                    bias=mx[:, ib, :],
                    accum_out=ssum[:, ib, :],
                )
            nc.vector.reciprocal(rsum[:], ssum[:])
            for ib in range(rb):
                nc.vector.tensor_scalar_mul(
                    sl_sb[:, ib, :], sl_sb[:, ib, :], rsum[:, ib, :]
                )

            # s shard out + AllGather
            nc.sync.dma_start(
                s_out.ap().rearrange("(b p) f -> p b f", p=128), sl_sb[:]
            )
            nc.gpsimd.dma_start(
                cc_s_in.ap().rearrange("(b p) f -> p b f", p=128), sl_sb[:]
            )
            nc.gpsimd.collective_compute(
                "AllGather",
                mybir.AluOpType.bypass,
                replica_groups=rgroups,
                ins=[cc_s_in.ap()],
                outs=[cc_s_out.ap()],
            )

            # ---- P3: AhatS_T accumulation over j blocks ----
            ast_ps = pacc.tile([128, rows], f32, tag="bigacc")
            for it in range(nb // 2):
                buf = strips.tile([128, 2, rows], f32, tag="abuf")
                nc.sync.dma_start(
                    mm(buf[:]),
                    at.ap()[it * 256 : (it + 1) * 256, :]
                    .rearrange("(t p) r -> p t r", p=128)
                    .bitcast(mmdt),
                )
                for t in range(2):
                    jb = 2 * it + t
                    sf = sfp.tile([128, F], f32, tag="sfull")
                    nc.sync.dma_start(
                        mm(sf[:]),
                        cc_s_out.ap()[jb * 128 : (jb + 1) * 128, :].bitcast(mmdt),
                    )
                    for c in range(nch):
                        nc.tensor.matmul(
                            ast_ps[:, c * ch : (c + 1) * ch],
                            mm(sf[:]),
                            mm(buf[:, t, c * ch : (c + 1) * ch]),
                            start=(jb == 0),
                            stop=(jb == nb - 1),
                        )
            for c in range(nch):
                nc.vector.tensor_copy(
                    ast_sb[:, c * ch : (c + 1) * ch], ast_ps[:, c * ch : (c + 1) * ch]
                )

            # As = AhatS_T.T - s_loc ; then final contractions
            xn_ps = pfin.tile([F, F], f32, tag="xnps")
            an_ps = pfin.tile([F, F], f32, tag="anps")
            for ib in range(rb):
                tr_ps = pmm.tile([128, 128], f32, tag="zps")
                nc.tensor.transpose(
                    tr_ps[:], ast_sb[:, ib * 128 : (ib + 1) * 128], ident[:]
                )
                nc.vector.tensor_sub(as_sb[:, ib, :], tr_ps[:], sl_sb[:, ib, :])
            for ib in range(rb):
                nc.tensor.matmul(
                    xn_ps[:],
                    sl_sb[:, ib, :],
                    z_sb[:, ib, :],
                    start=(ib == 0),
                    stop=(ib == rb - 1),
                )
                nc.tensor.matmul(
                    an_ps[:],
                    sl_sb[:, ib, :],
                    as_sb[:, ib, :],
                    start=(ib == 0),
                    stop=(ib == rb - 1),
                )
            nc.vector.tensor_copy(xn_sb[:], xn_ps[:])
            nc.vector.tensor_copy(an_sb[:], an_ps[:])
            nc.sync.dma_start(xn_out.ap(), xn_sb[:])
            nc.sync.dma_start(an_out.ap(), an_sb[:])

    nc.compile()
    return nc


def make_in_maps(X, A, W_embed, b_embed, W_assign, b_assign, n_cores=CORES_FULL):
    """Host-side sharding / layout prep. Returns one input dict per core."""
    X = np.ascontiguousarray(np.asarray(X, dtype=np.float32))
    A = np.asarray(A, dtype=np.float32)
    n = A.shape[0]
    rows = n // n_cores
    we_t = np.ascontiguousarray(np.asarray(W_embed, np.float32).T)
    wa_t = np.ascontiguousarray(np.asarray(W_assign, np.float32).T)
    be_b = np.ascontiguousarray(
        np.broadcast_to(np.asarray(b_embed, np.float32)[None, :], (128, F))
    )
    ba_b = np.ascontiguousarray(
        np.broadcast_to(np.asarray(b_assign, np.float32)[None, :], (128, F))
    )
    in_maps = []
    for c in range(n_cores):
        at = np.ascontiguousarray(A[c * rows : (c + 1) * rows, :].T)
        # bake A_hat = A + I into the shard
        at[np.arange(c * rows, (c + 1) * rows), np.arange(rows)] += 1.0
        in_maps.append(
            {
                "at_hat": at,
                "a_nat": np.ascontiguousarray(A[c * rows : (c + 1) * rows, :]),
                "x_in": X,
                "we_t": we_t,
                "wa_t": wa_t,
                "be_b": be_b,
                "ba_b": ba_b,
            }
        )
    return in_maps


_CACHE = {}


def _get_program(n, n_cores):
    key = (n, n_cores)
    if key not in _CACHE:
        _CACHE[key] = build_program(n, n_cores)
    return _CACHE[key]


def run_on_hw(inputs, n_cores=CORES_FULL, trace=False):
    """Compile (cached), run on hardware, return (outputs_tuple, BassKernelResults)."""
    from concourse.bass_interp import get_hw_module

    n = inputs["A"].shape[0]
    nc = _get_program(n, n_cores)
    in_maps = make_in_maps(n_cores=n_cores, **inputs)
    old_m = nc.m
    nc.m = get_hw_module(nc.m)
    try:
        res = run_bass_kernel_spmd(
            nc, in_maps, core_ids=list(range(n_cores)), trace=trace
        )
    finally:
        nc.m = old_m
    outs = res.results
    x_next = np.sum([r["xn_part"] for r in outs], axis=0, dtype=np.float32)
    a_next = np.sum([r["an_part"] for r in outs], axis=0, dtype=np.float32)
    s_l = np.concatenate([r["s_part"] for r in outs], axis=0)
    return (x_next, a_next, s_l), res


def kernel(X, A, W_embed, b_embed, W_assign, b_assign):
    out, _ = run_on_hw(
        dict(
            X=X,
            A=A,
            W_embed=W_embed,
            b_embed=b_embed,
            W_assign=W_assign,
            b_assign=b_assign,
        )
    )
    return out


# revision 32
# speedup vs baseline: 1.3319x; 1.1819x over previous
"""DiffPool forward (GCN embed/assign + pooled X/A) on 8 trn2 NeuronCores.

Sharding: 1D node partition. Core c owns rows R_c = [c*ROWS, (c+1)*ROWS).
Host feeds each core A_T_hat_c = (A[R_c,:] + I[R_c,:]).T  -> [N, ROWS], so the
contraction index j sits on the SBUF partition axis for both big products and
no transposes of A are ever done on device.

Device program (single SPMD NEFF, collectives across 8 cores):
  P1: deg_loc = column-sums of A_T_hat strips (DVE adds + partition tree)
      AllGather(deg) -> d = 1/sqrt(deg)
  P2: AHXT[f,i]   = sum_j (d*X)[j,f] * A_T_hat[j,i]   (PSUM accum over 64 j-blocks)
      z   = d_i * (AHXT.T @ We.T) + be
      s   = softmax(d_i * (AHXT.T @ Wa.T) + ba)       (row softmax, free axis)
      AllGather(s) -> s_full
  P3: AhatS_T[m,i] = sum_j s_full[j,m] * A_T_hat[j,i] (same strip pattern)
      As = AhatS_T.T - s_loc                          (undo baked +I)
      Xn_part = s_loc.T @ z ; An_part = s_loc.T @ As  (partials over local rows)
Host: X_next = sum_c Xn_part, A_next = sum_c An_part, s_l = concat_c(s_part).
"""

import ml_dtypes
import numpy as np

import concourse.bass as bass
import concourse.mybir as mybir
import concourse.tile as tile
from concourse import bacc
from concourse.bass_utils import run_bass_kernel_spmd
from concourse.masks import make_identity

F = 128  # F_IN == N_META == N_EMBED == 128
f32 = mybir.dt.float32
N_FULL = 8192
CORES_FULL = 8


def build_program(n=N_FULL, n_cores=CORES_FULL, fast32=True):
    """Build the SPMD single-core program (same NEFF on all cores)."""
    rows = n // n_cores      # local node rows per core
    nb = n // 128            # j blocks (global)
    rb = rows // 128         # local row blocks
    ch = min(512, rows)      # matmul moving-dim chunk
    nch = rows // ch         # chunks per strip
    assert nb % 2 == 0 and rows % 128 == 0
    mmdt = mybir.dt.float32r if fast32 else f32

    def mm(ap):
        return ap.bitcast(mmdt)

    nc = bacc.Bacc(
        "TRN2",
        target_bir_lowering=False,
        debug=False,
        enable_asserts=False,
        num_devices=n_cores,
    )

    bf16 = mybir.dt.bfloat16
    at = nc.dram_tensor("at_hat", [n, rows], f32, kind="ExternalInput")
    anat = nc.dram_tensor("a_nat", [rows, n], bf16, kind="ExternalInput")
    atb = nc.dram_tensor("at_bf16", [n, rows], bf16, kind="ExternalInput")
    x_in = nc.dram_tensor("x_in", [n, F], f32, kind="ExternalInput")
    wet = nc.dram_tensor("we_t", [F, F], f32, kind="ExternalInput")
    wat = nc.dram_tensor("wa_t", [F, F], f32, kind="ExternalInput")
    beb = nc.dram_tensor("be_b", [128, F], f32, kind="ExternalInput")
    bab = nc.dram_tensor("ba_b", [128, F], f32, kind="ExternalInput")

    s_out = nc.dram_tensor("s_part", [rows, F], f32, kind="ExternalOutput")
    xn_out = nc.dram_tensor("xn_part", [F, F], f32, kind="ExternalOutput")
    an_out = nc.dram_tensor("an_part", [F, F], f32, kind="ExternalOutput")

    cc_deg_in = nc.dram_tensor("cc_deg_in", [rows], f32, kind="Internal")
    cc_deg_out = nc.dram_tensor(
        "cc_deg_out", [n], f32, kind="Internal", addr_space="Shared"
    )
    cc_s_in = nc.dram_tensor("cc_s_in", [rows, F], bf16, kind="Internal")
    cc_s_out = nc.dram_tensor(
        "cc_s_out", [n, F], bf16, kind="Internal", addr_space="Shared"
    )
    rgroups = [list(range(n_cores))]

    cw = min(4096, n)        # P1 natural-tile column width
    nh = n // cw             # column halves per local row block

    with tile.TileContext(nc) as tc:
        with (
            tc.tile_pool(name="strips", bufs=8) as strips,
            tc.tile_pool(name="p3b", bufs=8) as p3b,
            tc.tile_pool(name="natb", bufs=3) as natb,
            tc.tile_pool(name="persist", bufs=1) as persist,
            tc.tile_pool(name="sf", bufs=4) as sfp,
            tc.tile_pool(name="pacc", bufs=1, space="PSUM") as pacc,
            tc.tile_pool(name="pmm", bufs=2, space="PSUM") as pmm,
            tc.tile_pool(name="pfin", bufs=1, space="PSUM") as pfin,
        ):
            # ---- persistent sbuf tensors ----
            x_sb = persist.tile([128, nb, F], f32)      # X, later d*X (j on partitions)
            degparts = persist.tile([128, rb, nh], f32)
            deg2d = persist.tile([128, rb], f32)
            d_sb = persist.tile([128, nb], f32)         # 1/sqrt(deg) full, [p, blk]
            d_rc = persist.tile([128, rb], f32)         # same for local rows
            wet_sb = persist.tile([F, F], f32)
            wat_sb = persist.tile([F, F], f32)
            beb_sb = persist.tile([128, F], f32)
            bab_sb = persist.tile([128, F], f32)
            ident = persist.tile([128, 128], f32)
            axt_sb = persist.tile([128, rows], f32)     # AHXT
            z_sb = persist.tile([128, rb, F], f32)
            sl_sb = persist.tile([128, rb, F], f32)     # logits -> s
            s_bf = persist.tile([128, rb, F], bf16)     # s in bf16 for P3
            as_sb = persist.tile([128, rb, F], f32)     # A @ s (local rows)
            ast_sb = persist.tile([128, rows], f32)     # AhatS_T
            mx = persist.tile([128, rb, 1], f32)
            ssum = persist.tile([128, rb, 1], f32)
            rsum = persist.tile([128, rb, 1], f32)
            xn_sb = persist.tile([F, F], f32)
            an_sb = persist.tile([F, F], f32)

            # ---- constant loads / setup ----
            nc.sync.dma_start(
                mm(x_sb[:]), x_in.ap().rearrange("(b p) f -> p b f", p=128).bitcast(mmdt)
            )
            nc.sync.dma_start(wet_sb[:], wet.ap())
            nc.sync.dma_start(wat_sb[:], wat.ap())
            nc.sync.dma_start(beb_sb[:], beb.ap())
            nc.sync.dma_start(bab_sb[:], bab.ap())
            make_identity(nc, ident[:])

            # ---- P1: degrees = row sums of the natural-layout shard ----
            for rbi in range(rb):
                for h in range(nh):
                    t = natb.tile([128, cw], bf16, tag="nbuf")
                    nc.sync.dma_start(
                        t[:],
                        anat.ap()[rbi * 128 : (rbi + 1) * 128, h * cw : (h + 1) * cw],
                    )
                    nc.vector.reduce_sum(
                        degparts[:, rbi, h : h + 1], t[:], axis=mybir.AxisListType.X
                    )
            # combine halves and add the A_hat self-loop (+1)
            if nh == 2:
                nc.vector.scalar_tensor_tensor(
                    deg2d[:],
                    degparts[:, :, 0],
                    1.0,
                    degparts[:, :, 1],
                    op0=mybir.AluOpType.add,
                    op1=mybir.AluOpType.add,
                )
            else:
                assert nh == 1
                nc.vector.tensor_scalar_add(deg2d[:], degparts[:, :, 0], 1.0)

            # AllGather degrees; local d read back from the DRAM bounce so no
            # core-dependent indexing is needed anywhere in the program
            nc.gpsimd.dma_start(
                cc_deg_in.ap().rearrange("(b p) -> p b", p=128), deg2d[:]
            )
            nc.sync.dma_start(
                d_rc[:], cc_deg_in.ap().rearrange("(b p) -> p b", p=128)
            )
            nc.scalar.sqrt(d_rc[:], d_rc[:])
            nc.vector.reciprocal(d_rc[:], d_rc[:])
            nc.gpsimd.collective_compute(
                "AllGather",
                mybir.AluOpType.bypass,
                replica_groups=rgroups,
                ins=[cc_deg_in.ap()],
                outs=[cc_deg_out.ap()],
            )
            nc.sync.dma_start(
                d_sb[:], cc_deg_out.ap().rearrange("(b p) -> p b", p=128)
            )
            nc.scalar.sqrt(d_sb[:], d_sb[:])
            nc.vector.reciprocal(d_sb[:], d_sb[:])

            # X <- d * X (rows j scaled by d[j]); written as f32r for the PE
            for b in range(nb):
                nc.vector.tensor_scalar_mul(
                    mm(x_sb[:, b, :]), x_sb[:, b, :], d_sb[:, b : b + 1]
                )

            # ---- P2: AHXT accumulation over j blocks ----
            axt_ps = pacc.tile([128, rows], f32, tag="bigacc")
            for it in range(nb // 2):
                buf = strips.tile([128, 2, rows], f32, tag="abuf")
                nc.sync.dma_start(
                    mm(buf[:]),
                    at.ap()[it * 256 : (it + 1) * 256, :]
                    .rearrange("(t p) r -> p t r", p=128)
                    .bitcast(mmdt),
                )
                for t in range(2):
                    jb = 2 * it + t
                    for c in range(nch):
                        nc.tensor.matmul(
                            axt_ps[:, c * ch : (c + 1) * ch],
                            mm(x_sb[:, jb, :]),
                            mm(buf[:, t, c * ch : (c + 1) * ch]),
                            start=(jb == 0),
                            stop=(jb == nb - 1),
                        )
            for c in range(nch):
                nc.vector.tensor_copy(
                    axt_sb[:, c * ch : (c + 1) * ch], axt_ps[:, c * ch : (c + 1) * ch]
                )

            # ---- z, softmax(s) per local row block ----
            for ib in range(rb):
                z_ps = pmm.tile([128, F], f32, tag="zps")
                nc.tensor.matmul(
                    z_ps[:], axt_sb[:, ib * 128 : (ib + 1) * 128], wet_sb[:]
                )
                nc.vector.scalar_tensor_tensor(
                    z_sb[:, ib, :],
                    z_ps[:],
                    d_rc[:, ib : ib + 1],
                    beb_sb[:],
                    op0=mybir.AluOpType.mult,
                    op1=mybir.AluOpType.add,
                )
                sl_ps = pmm.tile([128, F], f32, tag="zps")
                nc.tensor.matmul(
                    sl_ps[:], axt_sb[:, ib * 128 : (ib + 1) * 128], wat_sb[:]
                )
                nc.vector.scalar_tensor_tensor(
                    sl_sb[:, ib, :],
                    sl_ps[:],
                    d_rc[:, ib : ib + 1],
                    bab_sb[:],
                    op0=mybir.AluOpType.mult,
                    op1=mybir.AluOpType.add,
                )
            nc.vector.reduce_max(
                mx[:], sl_sb[:], axis=mybir.AxisListType.X, negate=True
            )
            for ib in range(rb):
                nc.scalar.activation(
                    sl_sb[:, ib, :],
                    sl_sb[:, ib, :],
                    mybir.ActivationFunctionType.Exp,
                    bias=mx[:, ib, :],
                    accum_out=ssum[:, ib, :],
                )
            nc.vector.reciprocal(rsum[:], ssum[:])
            for ib in range(rb):
                nc.vector.tensor_scalar_mul(
                    sl_sb[:, ib, :], sl_sb[:, ib, :], rsum[:, ib, :]
                )

            # s shard out + AllGather (bf16 payload) + hoisted X_next partial
            xn_ps = pfin.tile([F, F], f32, tag="xnps")
            nc.sync.dma_start(
                s_out.ap().rearrange("(b p) f -> p b f", p=128), sl_sb[:]
            )
            for ib in range(rb):
                nc.vector.tensor_copy(s_bf[:, ib, :], sl_sb[:, ib, :])
            nc.gpsimd.dma_start(
                cc_s_in.ap().rearrange("(b p) f -> p b f", p=128), s_bf[:]
            )
            nc.gpsimd.collective_compute(
                "AllGather",
                mybir.AluOpType.bypass,
                replica_groups=rgroups,
                ins=[cc_s_in.ap()],
                outs=[cc_s_out.ap()],
            )
            for ib in range(rb):
                nc.tensor.matmul(
                    xn_ps[:],
                    sl_sb[:, ib, :],
                    z_sb[:, ib, :],
                    start=(ib == 0),
                    stop=(ib == rb - 1),
                )

            # ---- P3: AhatS_T accumulation over j blocks ----
            ast_ps = pacc.tile([128, rows], f32, tag="bigacc")
            for it in range(nb // 2):
                buf = p3b.tile([128, 2, rows], bf16, tag="bbuf")
                nc.sync.dma_start(
                    buf[:],
                    atb.ap()[it * 256 : (it + 1) * 256, :].rearrange(
                        "(t p) r -> p t r", p=128
                    ),
                )
                for t in range(2):
                    jb = 2 * it + t
                    sf = sfp.tile([128, F], bf16, tag="sfull")
                    nc.sync.dma_start(
                        sf[:], cc_s_out.ap()[jb * 128 : (jb + 1) * 128, :]
                    )
                    for c in range(nch):
                        nc.tensor.matmul(
                            ast_ps[:, c * ch : (c + 1) * ch],
                            sf[:],
                            buf[:, t, c * ch : (c + 1) * ch],
                            start=(jb == 0),
                            stop=(jb == nb - 1),
                        )
            for c in range(nch):
                nc.vector.tensor_copy(
                    ast_sb[:, c * ch : (c + 1) * ch], ast_ps[:, c * ch : (c + 1) * ch]
                )

            # As = AhatS_T.T - s_loc ; then final contraction
            an_ps = pfin.tile([F, F], f32, tag="anps")
            for ib in range(rb):
                tr_ps = pmm.tile([128, 128], f32, tag="zps")
                nc.tensor.transpose(
                    tr_ps[:], ast_sb[:, ib * 128 : (ib + 1) * 128], ident[:]
                )
                nc.vector.tensor_sub(as_sb[:, ib, :], tr_ps[:], sl_sb[:, ib, :])
            for ib in range(rb):
                nc.tensor.matmul(
                    an_ps[:],
                    sl_sb[:, ib, :],
                    as_sb[:, ib, :],
                    start=(ib == 0),
                    stop=(ib == rb - 1),
                )
            nc.vector.tensor_copy(xn_sb[:], xn_ps[:])
            nc.vector.tensor_copy(an_sb[:], an_ps[:])
            nc.sync.dma_start(xn_out.ap(), xn_sb[:])
            nc.sync.dma_start(an_out.ap(), an_sb[:])

    nc.compile()
    return nc


def make_in_maps(X, A, W_embed, b_embed, W_assign, b_assign, n_cores=CORES_FULL):
    """Host-side sharding / layout prep. Returns one input dict per core."""
    X = np.ascontiguousarray(np.asarray(X, dtype=np.float32))
    A = np.asarray(A, dtype=np.float32)
    n = A.shape[0]
    rows = n // n_cores
    we_t = np.ascontiguousarray(np.asarray(W_embed, np.float32).T)
    wa_t = np.ascontiguousarray(np.asarray(W_assign, np.float32).T)
    be_b = np.ascontiguousarray(
        np.broadcast_to(np.asarray(b_embed, np.float32)[None, :], (128, F))
    )
    ba_b = np.ascontiguousarray(
        np.broadcast_to(np.asarray(b_assign, np.float32)[None, :], (128, F))
    )
    in_maps = []
    for c in range(n_cores):
        at = np.ascontiguousarray(A[c * rows : (c + 1) * rows, :].T)
        # bake A_hat = A + I into the shard
        at[np.arange(c * rows, (c + 1) * rows), np.arange(rows)] += 1.0
        in_maps.append(
            {
                "at_hat": at,
                "at_bf16": at.astype(ml_dtypes.bfloat16),
                "a_nat": np.ascontiguousarray(
                    A[c * rows : (c + 1) * rows, :]
                ).astype(ml_dtypes.bfloat16),
                "x_in": X,
                "we_t": we_t,
                "wa_t": wa_t,
                "be_b": be_b,
                "ba_b": ba_b,
            }
        )
    return in_maps


_CACHE = {}


def _get_program(n, n_cores):
    key = (n, n_cores)
    if key not in _CACHE:
        _CACHE[key] = build_program(n, n_cores)
    return _CACHE[key]


def run_on_hw(inputs, n_cores=CORES_FULL, trace=False):
    """Compile (cached), run on hardware, return (outputs_tuple, BassKernelResults)."""
    from concourse.bass_interp import get_hw_module

    n = inputs["A"].shape[0]
    nc = _get_program(n, n_cores)
    in_maps = make_in_maps(n_cores=n_cores, **inputs)
    old_m = nc.m
    nc.m = get_hw_module(nc.m)
    try:
        res = run_bass_kernel_spmd(
            nc, in_maps, core_ids=list(range(n_cores)), trace=trace
        )
    finally:
        nc.m = old_m
    outs = res.results
    x_next = np.sum([r["xn_part"] for r in outs], axis=0, dtype=np.float32)
    a_next = np.sum([r["an_part"] for r in outs], axis=0, dtype=np.float32)
    s_l = np.concatenate([r["s_part"] for r in outs], axis=0)
    return (x_next, a_next, s_l), res


def kernel(X, A, W_embed, b_embed, W_assign, b_assign):
    out, _ = run_on_hw(
        dict(
            X=X,
            A=A,
            W_embed=W_embed,
            b_embed=b_embed,
            W_assign=W_assign,
            b_assign=b_assign,
        )
    )
    return out


# revision 42
# speedup vs baseline: 1.3470x; 1.0113x over previous
"""DiffPool forward (GCN embed/assign + pooled X/A) on 8 trn2 NeuronCores.

Sharding: 1D node partition. Core c owns rows R_c = [c*ROWS, (c+1)*ROWS).
Host feeds each core A_T_hat_c = (A[R_c,:] + I[R_c,:]).T  -> [N, ROWS], so the
contraction index j sits on the SBUF partition axis for both big products and
no transposes of A are ever done on device.

Device program (single SPMD NEFF, collectives across 8 cores):
  P1: deg_loc = column-sums of A_T_hat strips (DVE adds + partition tree)
      AllGather(deg) -> d = 1/sqrt(deg)
  P2: AHXT[f,i]   = sum_j (d*X)[j,f] * A_T_hat[j,i]   (PSUM accum over 64 j-blocks)
      z   = d_i * (AHXT.T @ We.T) + be
      s   = softmax(d_i * (AHXT.T @ Wa.T) + ba)       (row softmax, free axis)
      AllGather(s) -> s_full
  P3: AhatS_T[m,i] = sum_j s_full[j,m] * A_T_hat[j,i] (same strip pattern)
      As = AhatS_T.T - s_loc                          (undo baked +I)
      Xn_part = s_loc.T @ z ; An_part = s_loc.T @ As  (partials over local rows)
Host: X_next = sum_c Xn_part, A_next = sum_c An_part, s_l = concat_c(s_part).
"""

import numpy as np

import concourse.bass as bass
import concourse.mybir as mybir
import concourse.tile as tile
from concourse import bacc
from concourse.bass_utils import run_bass_kernel_spmd
from concourse.masks import make_identity

F = 128  # F_IN == N_META == N_EMBED == 128
f32 = mybir.dt.float32
N_FULL = 8192
CORES_FULL = 8


def build_program(n=N_FULL, n_cores=CORES_FULL, fast32=True):
    """Build the SPMD single-core program (same NEFF on all cores)."""
    rows = n // n_cores      # local node rows per core
    nb = n // 128            # j blocks (global)
    rb = rows // 128         # local row blocks
    ch = min(512, rows)      # matmul moving-dim chunk
    nch = rows // ch         # chunks per strip
    assert nb % 2 == 0 and rows % 128 == 0
    mmdt = mybir.dt.float32r if fast32 else f32

    def mm(ap):
        return ap.bitcast(mmdt)

    nc = bacc.Bacc(
        "TRN2",
        target_bir_lowering=False,
        debug=False,
        enable_asserts=False,
        num_devices=n_cores,
    )

    f16 = mybir.dt.float16
    at = nc.dram_tensor("at_hat", [n, rows], f32, kind="ExternalInput")
    atb = nc.dram_tensor("at_f16", [n, rows], f16, kind="ExternalInput")
    x_in = nc.dram_tensor("x_in", [n, F], f32, kind="ExternalInput")
    wet = nc.dram_tensor("we_t", [F, F], f32, kind="ExternalInput")
    wat = nc.dram_tensor("wa_t", [F, F], f32, kind="ExternalInput")
    beb = nc.dram_tensor("be_b", [128, F], f32, kind="ExternalInput")
    bab = nc.dram_tensor("ba_b", [128, F], f32, kind="ExternalInput")

    s_out = nc.dram_tensor("s_part", [rows, F], f32, kind="ExternalOutput")
    xn_out = nc.dram_tensor("xn_part", [F, F], f32, kind="ExternalOutput")
    an_out = nc.dram_tensor("an_part", [F, F], f32, kind="ExternalOutput")

    cc_deg_in = nc.dram_tensor("cc_deg_in", [rows], f32, kind="Internal")
    cc_deg_out = nc.dram_tensor(
        "cc_deg_out", [n], f32, kind="Internal", addr_space="Shared"
    )
    cc_s_in = nc.dram_tensor("cc_s_in", [rows, F], f16, kind="Internal")
    cc_s_out = nc.dram_tensor(
        "cc_s_out", [n, F], f16, kind="Internal", addr_space="Shared"
    )
    rgroups = [list(range(n_cores))]

    with tile.TileContext(nc) as tc:
        with (
            tc.tile_pool(name="strips", bufs=8) as strips,
            tc.tile_pool(name="p3b", bufs=12) as p3b,
            tc.tile_pool(name="persist", bufs=1) as persist,
            tc.tile_pool(name="sf", bufs=4) as sfp,
            tc.tile_pool(name="pacc", bufs=1, space="PSUM") as pacc,
            tc.tile_pool(name="pmm", bufs=2, space="PSUM") as pmm,
            tc.tile_pool(name="pfin", bufs=1, space="PSUM") as pfin,
        ):
            # ---- persistent sbuf tensors ----
            x_sb = persist.tile([128, nb, F], f32)      # X, later d*X (j on partitions)
            ones16 = persist.tile([128, 1], f16)
            deg_line = persist.tile([1, rows], f32)
            d_sb = persist.tile([128, nb], f32)         # 1/sqrt(deg) full, [p, blk]
            d_rc = persist.tile([128, rb], f32)         # same for local rows
            wet_sb = persist.tile([F, F], f32)
            wat_sb = persist.tile([F, F], f32)
            beb_sb = persist.tile([128, F], f32)
            bab_sb = persist.tile([128, F], f32)
            ident = persist.tile([128, 128], f32)
            axt_sb = persist.tile([128, rows], f32)     # AHXT
            z_sb = persist.tile([128, rb, F], f32)
            sl_sb = persist.tile([128, rb, F], f32)     # logits -> s
            s_bf = persist.tile([128, rb, F], f16)      # s in fp16 for P3
            as_sb = persist.tile([128, rb, F], f32)     # A @ s (local rows)
            ast_sb = persist.tile([128, rows], f32)     # AhatS_T
            mx = persist.tile([128, rb, 1], f32)
            ssum = persist.tile([128, rb, 1], f32)
            rsum = persist.tile([128, rb, 1], f32)
            xn_sb = persist.tile([F, F], f32)
            an_sb = persist.tile([F, F], f32)

            # ---- constant loads / setup ----
            nc.sync.dma_start(
                mm(x_sb[:]), x_in.ap().rearrange("(b p) f -> p b f", p=128).bitcast(mmdt)
            )
            nc.sync.dma_start(wet_sb[:], wet.ap())
            nc.sync.dma_start(wat_sb[:], wat.ap())
            nc.sync.dma_start(beb_sb[:], beb.ap())
            nc.sync.dma_start(bab_sb[:], bab.ap())
            make_identity(nc, ident[:])
            nc.vector.memset(ones16[:], 1.0)

            # ---- P1: degrees = column sums of the fp16 transposed shard,
            # computed as ones-vector matmuls on the otherwise-idle PE.
            # (at_f16 has A_hat baked in, so the +1 self-loop is included.)
            deg_pss = [
                pmm.tile([1, ch], f32, tag="degps", name=f"deg_ps{c}")
                for c in range(nch)
            ]
            for it in range(nb // 2):
                buf = p3b.tile([128, 2, rows], f16, tag="bbuf")
                nc.sync.dma_start(
                    buf[:],
                    atb.ap()[it * 256 : (it + 1) * 256, :].rearrange(
                        "(t p) r -> p t r", p=128
                    ),
                )
                for t in range(2):
                    jb = 2 * it + t
                    for c in range(nch):
                        nc.tensor.matmul(
                            deg_pss[c][:],
                            ones16[:],
                            buf[:, t, c * ch : (c + 1) * ch],
                            start=(jb == 0),
                            stop=(jb == nb - 1),
                        )
            for c in range(nch):
                nc.vector.tensor_copy(
                    deg_line[:, c * ch : (c + 1) * ch], deg_pss[c][:]
                )

            # AllGather degrees; local d read back from the DRAM bounce so no
            # core-dependent indexing is needed anywhere in the program
            nc.gpsimd.dma_start(
                cc_deg_in.ap().rearrange("(a b) -> a b", a=1), deg_line[:]
            )
            nc.sync.dma_start(
                d_rc[:], cc_deg_in.ap().rearrange("(b p) -> p b", p=128)
            )
            nc.scalar.sqrt(d_rc[:], d_rc[:])
            nc.vector.reciprocal(d_rc[:], d_rc[:])
            nc.gpsimd.collective_compute(
                "AllGather",
                mybir.AluOpType.bypass,
                replica_groups=rgroups,
                ins=[cc_deg_in.ap()],
                outs=[cc_deg_out.ap()],
            )
            nc.sync.dma_start(
                d_sb[:], cc_deg_out.ap().rearrange("(b p) -> p b", p=128)
            )
            nc.scalar.sqrt(d_sb[:], d_sb[:])
            nc.vector.reciprocal(d_sb[:], d_sb[:])

            # X <- d * X (rows j scaled by d[j]); written as f32r for the PE
            nc.vector.tensor_tensor(
                mm(x_sb[:]),
                x_sb[:],
                d_sb[:].broadcast_to((128, nb, F)),
                op=mybir.AluOpType.mult,
            )

            # ---- P2: AHXT accumulation over j blocks ----
            axt_ps = pacc.tile([128, rows], f32, tag="bigacc")
            for it in range(nb // 2):
                buf = strips.tile([128, 2, rows], f32, tag="abuf")
                nc.sync.dma_start(
                    mm(buf[:]),
                    at.ap()[it * 256 : (it + 1) * 256, :]
                    .rearrange("(t p) r -> p t r", p=128)
                    .bitcast(mmdt),
                )
                for t in range(2):
                    jb = 2 * it + t
                    for c in range(nch):
                        nc.tensor.matmul(
                            axt_ps[:, c * ch : (c + 1) * ch],
                            mm(x_sb[:, jb, :]),
                            mm(buf[:, t, c * ch : (c + 1) * ch]),
                            start=(jb == 0),
                            stop=(jb == nb - 1),
                        )
            for c in range(nch):
                nc.vector.tensor_copy(
                    axt_sb[:, c * ch : (c + 1) * ch], axt_ps[:, c * ch : (c + 1) * ch]
                )

            # ---- z, softmax(s), grouped in PSUM-bank-sized batches ----
            grp = min(4, rb)
            for g in range(rb // grp):
                lo, hi = g * grp, (g + 1) * grp
                z_ps = pmm.tile([128, grp, F], f32, tag="zps")
                sl_ps = pmm.tile([128, grp, F], f32, tag="zps")
                for k in range(grp):
                    ib = lo + k
                    nc.tensor.matmul(
                        z_ps[:, k, :], axt_sb[:, ib * 128 : (ib + 1) * 128], wet_sb[:]
                    )
                    nc.tensor.matmul(
                        sl_ps[:, k, :], axt_sb[:, ib * 128 : (ib + 1) * 128], wat_sb[:]
                    )
                d_bc = d_rc[:, lo:hi].broadcast_to((128, grp, F))
                nc.vector.tensor_tensor(
                    z_sb[:, lo:hi, :], z_ps[:], d_bc, op=mybir.AluOpType.mult
                )
                nc.vector.tensor_tensor(
                    sl_sb[:, lo:hi, :], sl_ps[:], d_bc, op=mybir.AluOpType.mult
                )
            bias_e = beb_sb[:].broadcast_to((128, F, rb)).rearrange("p f b -> p b f")
            bias_a = bab_sb[:].broadcast_to((128, F, rb)).rearrange("p f b -> p b f")
            nc.vector.tensor_tensor(z_sb[:], z_sb[:], bias_e, op=mybir.AluOpType.add)
            nc.vector.tensor_tensor(sl_sb[:], sl_sb[:], bias_a, op=mybir.AluOpType.add)
            nc.vector.reduce_max(
                mx[:], sl_sb[:], axis=mybir.AxisListType.X, negate=True
            )
            for ib in range(rb):
                nc.scalar.activation(
                    sl_sb[:, ib, :],
                    sl_sb[:, ib, :],
                    mybir.ActivationFunctionType.Exp,
                    bias=mx[:, ib, :],
                    accum_out=ssum[:, ib, :],
                )
            nc.vector.reciprocal(rsum[:], ssum[:])
            nc.vector.tensor_tensor(
                sl_sb[:],
                sl_sb[:],
                rsum[:].broadcast_to((128, rb, F)),
                op=mybir.AluOpType.mult,
            )

            # s shard out + AllGather (bf16 payload) + hoisted X_next partial
            xn_ps = pfin.tile([F, F], f32, tag="xnps")
            nc.sync.dma_start(
                s_out.ap().rearrange("(b p) f -> p b f", p=128), sl_sb[:]
            )
            nc.vector.tensor_copy(s_bf[:], sl_sb[:])
            nc.gpsimd.dma_start(
                cc_s_in.ap().rearrange("(b p) f -> p b f", p=128), s_bf[:]
            )
            nc.gpsimd.collective_compute(
                "AllGather",
                mybir.AluOpType.bypass,
                replica_groups=rgroups,
                ins=[cc_s_in.ap()],
                outs=[cc_s_out.ap()],
            )
            for ib in range(rb):
                nc.tensor.matmul(
                    xn_ps[:],
                    sl_sb[:, ib, :],
                    z_sb[:, ib, :],
                    start=(ib == 0),
                    stop=(ib == rb - 1),
                )

            # ---- P3: AhatS_T accumulation over j blocks ----
            ast_ps = pacc.tile([128, rows], f32, tag="bigacc")
            for it in range(nb // 2):
                buf = p3b.tile([128, 2, rows], f16, tag="bbuf")
                nc.sync.dma_start(
                    buf[:],
                    atb.ap()[it * 256 : (it + 1) * 256, :].rearrange(
                        "(t p) r -> p t r", p=128
                    ),
                )
                for t in range(2):
                    jb = 2 * it + t
                    sf = sfp.tile([128, F], f16, tag="sfull")
                    nc.sync.dma_start(
                        sf[:], cc_s_out.ap()[jb * 128 : (jb + 1) * 128, :]
                    )
                    for c in range(nch):
                        nc.tensor.matmul(
                            ast_ps[:, c * ch : (c + 1) * ch],
                            sf[:],
                            buf[:, t, c * ch : (c + 1) * ch],
                            start=(jb == 0),
                            stop=(jb == nb - 1),
                        )
            for c in range(nch):
                nc.vector.tensor_copy(
                    ast_sb[:, c * ch : (c + 1) * ch], ast_ps[:, c * ch : (c + 1) * ch]
                )

            # As = AhatS_T.T - s_loc ; then final contraction
            an_ps = pfin.tile([F, F], f32, tag="anps")
            for ib in range(rb):
                tr_ps = pmm.tile([128, 128], f32, tag="zps")
                nc.tensor.transpose(
                    tr_ps[:], ast_sb[:, ib * 128 : (ib + 1) * 128], ident[:]
                )
                nc.vector.tensor_sub(as_sb[:, ib, :], tr_ps[:], sl_sb[:, ib, :])
            for ib in range(rb):
                nc.tensor.matmul(
                    an_ps[:],
                    sl_sb[:, ib, :],
                    as_sb[:, ib, :],
                    start=(ib == 0),
                    stop=(ib == rb - 1),
                )
            nc.vector.tensor_copy(xn_sb[:], xn_ps[:])
            nc.vector.tensor_copy(an_sb[:], an_ps[:])
            nc.sync.dma_start(xn_out.ap(), xn_sb[:])
            nc.sync.dma_start(an_out.ap(), an_sb[:])

    nc.compile()
    return nc


def make_in_maps(X, A, W_embed, b_embed, W_assign, b_assign, n_cores=CORES_FULL):
    """Host-side sharding / layout prep. Returns one input dict per core."""
    X = np.ascontiguousarray(np.asarray(X, dtype=np.float32))
    A = np.asarray(A, dtype=np.float32)
    n = A.shape[0]
    rows = n // n_cores
    we_t = np.ascontiguousarray(np.asarray(W_embed, np.float32).T)
    wa_t = np.ascontiguousarray(np.asarray(W_assign, np.float32).T)
    be_b = np.ascontiguousarray(
        np.broadcast_to(np.asarray(b_embed, np.float32)[None, :], (128, F))
    )
    ba_b = np.ascontiguousarray(
        np.broadcast_to(np.asarray(b_assign, np.float32)[None, :], (128, F))
    )
    in_maps = []
    for c in range(n_cores):
        at = np.ascontiguousarray(A[c * rows : (c + 1) * rows, :].T)
        # bake A_hat = A + I into the shard
        at[np.arange(c * rows, (c + 1) * rows), np.arange(rows)] += 1.0
        in_maps.append(
            {
                "at_hat": at,
                "at_f16": at.astype(np.float16),
                "x_in": X,
                "we_t": we_t,
                "wa_t": wa_t,
                "be_b": be_b,
                "ba_b": ba_b,
            }
        )
    return in_maps


_CACHE = {}


def _get_program(n, n_cores):
    key = (n, n_cores)
    if key not in _CACHE:
        _CACHE[key] = build_program(n, n_cores)
    return _CACHE[key]


def run_on_hw(inputs, n_cores=CORES_FULL, trace=False):
    """Compile (cached), run on hardware, return (outputs_tuple, BassKernelResults)."""
    from concourse.bass_interp import get_hw_module

    n = inputs["A"].shape[0]
    nc = _get_program(n, n_cores)
    in_maps = make_in_maps(n_cores=n_cores, **inputs)
    old_m = nc.m
    nc.m = get_hw_module(nc.m)
    try:
        res = run_bass_kernel_spmd(
            nc, in_maps, core_ids=list(range(n_cores)), trace=trace
        )
    finally:
        nc.m = old_m
    outs = res.results
    x_next = np.sum([r["xn_part"] for r in outs], axis=0, dtype=np.float32)
    a_next = np.sum([r["an_part"] for r in outs], axis=0, dtype=np.float32)
    s_l = np.concatenate([r["s_part"] for r in outs], axis=0)
    return (x_next, a_next, s_l), res


def kernel(X, A, W_embed, b_embed, W_assign, b_assign):
    out, _ = run_on_hw(
        dict(
            X=X,
            A=A,
            W_embed=W_embed,
            b_embed=b_embed,
            W_assign=W_assign,
            b_assign=b_assign,
        )
    )
    return out


# revision 48
# speedup vs baseline: 1.3984x; 1.0381x over previous
"""DiffPool forward (GCN embed/assign + pooled X/A) on 8 trn2 NeuronCores.

Sharding: 1D node partition. Core c owns rows R_c = [c*ROWS, (c+1)*ROWS).
Host feeds each core A_T_hat_c = (A[R_c,:] + I[R_c,:]).T  -> [N, ROWS], so the
contraction index j sits on the SBUF partition axis for both big products and
no transposes of A are ever done on device.

Device program (single SPMD NEFF, collectives across 8 cores):
  P1: deg_loc = column-sums of fp16 A_T_hat strips via ones-vector matmuls on
      the otherwise-idle PE (4-way tile_position col packing). The tail strip
      buffers stay resident in SBUF for P3 to reuse.
      AllGather(deg) -> d = 1/sqrt(deg)
  P2: AHXT[f,i]   = sum_j (d*X)[j,f] * A_T_hat[j,i]   (fp32r PSUM accum, f32 strips)
      z   = d_i * (AHXT.T @ We.T) + be
      s   = softmax(d_i * (AHXT.T @ Wa.T) + ba)       (row softmax, free axis)
      AllGather(s in fp16) -> s_full
  P3: AhatS_T[m,i] = sum_j s_full[j,m] * A_T_hat[j,i] (fp16 strips, reversed j
      order so P1's resident buffers are hit first)
      As = AhatS_T.T - s_loc                          (undo baked +I)
      Xn_part = s_loc.T @ z ; An_part = s_loc.T @ As  (partials over local rows)
Host: X_next = sum_c Xn_part, A_next = sum_c An_part, s_l = concat_c(s_part).

Precision: P2 (mixed-sign contraction) keeps fp32-width data (fp32r matmul
mode, ~1e-5 rel err). deg and A@s are nonnegative sums of ~8192 terms, so
independent fp16 input-rounding attenuates by ~1/sqrt(N): fp16 there costs
~1e-5..1e-4 rel err on A_next only, and halves those passes' DMA.
"""

import numpy as np

import concourse.bass as bass
import concourse.mybir as mybir
import concourse.tile as tile
from concourse import bacc
from concourse.bass_utils import run_bass_kernel_spmd
from concourse.masks import make_identity

F = 128  # F_IN == N_META == N_EMBED == 128
f32 = mybir.dt.float32
N_FULL = 8192
CORES_FULL = 8


def build_program(n=N_FULL, n_cores=CORES_FULL, fast32=True):
    """Build the SPMD single-core program (same NEFF on all cores)."""
    rows = n // n_cores      # local node rows per core
    nb = n // 128            # j blocks (global)
    rb = rows // 128         # local row blocks
    ch = min(512, rows)      # matmul moving-dim chunk
    nch = rows // ch         # chunks per strip
    assert nb % 2 == 0 and rows % 128 == 0
    mmdt = mybir.dt.float32r if fast32 else f32

    def mm(ap):
        return ap.bitcast(mmdt)

    nc = bacc.Bacc(
        "TRN2",
        target_bir_lowering=False,
        debug=False,
        enable_asserts=False,
        num_devices=n_cores,
    )

    f16 = mybir.dt.float16
    at = nc.dram_tensor("at_hat", [n, rows], f32, kind="ExternalInput")
    atb = nc.dram_tensor("at_f16", [n, rows], f16, kind="ExternalInput")
    x_in = nc.dram_tensor("x_in", [n, F], f32, kind="ExternalInput")
    wet = nc.dram_tensor("we_t", [F, F], f32, kind="ExternalInput")
    wat = nc.dram_tensor("wa_t", [F, F], f32, kind="ExternalInput")
    beb = nc.dram_tensor("be_b", [128, F], f32, kind="ExternalInput")
    bab = nc.dram_tensor("ba_b", [128, F], f32, kind="ExternalInput")

    s_out = nc.dram_tensor("s_part", [rows, F], f32, kind="ExternalOutput")
    xn_out = nc.dram_tensor("xn_part", [F, F], f32, kind="ExternalOutput")
    an_out = nc.dram_tensor("an_part", [F, F], f32, kind="ExternalOutput")

    cc_deg_in = nc.dram_tensor("cc_deg_in", [rows], f32, kind="Internal")
    cc_deg_out = nc.dram_tensor(
        "cc_deg_out", [n], f32, kind="Internal", addr_space="Shared"
    )
    cc_s_in = nc.dram_tensor("cc_s_in", [rows, F], f16, kind="Internal")
    cc_s_out = nc.dram_tensor(
        "cc_s_out", [n, F], f16, kind="Internal", addr_space="Shared"
    )
    rgroups = [list(range(n_cores))]

    nres = min(12, nb // 2)  # P1 tail buffers kept resident for P3 reuse

    with tile.TileContext(nc) as tc:
        with (
            tc.tile_pool(name="strips", bufs=7) as strips,
            tc.tile_pool(name="p3b", bufs=5) as p3b,
            tc.tile_pool(name="p3res", bufs=nres) as p3res,
            tc.tile_pool(name="persist", bufs=1) as persist,
            tc.tile_pool(name="sf", bufs=4) as sfp,
            tc.tile_pool(name="pacc", bufs=1, space="PSUM") as pacc,
            tc.tile_pool(name="pmm", bufs=2, space="PSUM") as pmm,
            tc.tile_pool(name="pfin", bufs=1, space="PSUM") as pfin,
        ):
            # ---- persistent sbuf tensors ----
            x_sb = persist.tile([128, nb, F], f32)      # X, later d*X (j on partitions)
            ones16 = persist.tile([128, 1], f16)
            deg_line = persist.tile([1, rows], f32)
            d_sb = persist.tile([128, nb], f32)         # 1/sqrt(deg) full, [p, blk]
            d_rc = persist.tile([128, rb], f32)         # same for local rows
            wet_sb = persist.tile([F, F], f32)
            wat_sb = persist.tile([F, F], f32)
            beb_sb = persist.tile([128, F], f32)
            bab_sb = persist.tile([128, F], f32)
            ident = persist.tile([128, 128], f32)
            axt_sb = persist.tile([128, rows], f32)     # AHXT
            z_sb = persist.tile([128, rb, F], f32)
            sl_sb = persist.tile([128, rb, F], f32)     # logits -> s
            s_bf = persist.tile([128, rb, F], f16)      # s in fp16 for P3
            as_sb = persist.tile([128, rb, F], f32)     # A @ s (local rows)
            ast_sb = persist.tile([128, rows], f32)     # AhatS_T
            mx = persist.tile([128, rb, 1], f32)
            ssum = persist.tile([128, rb, 1], f32)
            rsum = persist.tile([128, rb, 1], f32)
            xn_sb = persist.tile([F, F], f32)
            an_sb = persist.tile([F, F], f32)

            # ---- constant loads / setup ----
            # nc.sync is reserved for the big strip streams; small loads go on
            # gpsimd/scalar so a dependent DMA never blocks strip prefetch.
            nc.sync.dma_start(
                mm(x_sb[:]), x_in.ap().rearrange("(b p) f -> p b f", p=128).bitcast(mmdt)
            )
            nc.gpsimd.dma_start(wet_sb[:], wet.ap())
            nc.gpsimd.dma_start(wat_sb[:], wat.ap())
            nc.gpsimd.dma_start(beb_sb[:], beb.ap())
            nc.gpsimd.dma_start(bab_sb[:], bab.ap())
            make_identity(nc, ident[:])
            nc.vector.memset(ones16[:], 1.0)

            # ---- P1: degrees = column sums of the fp16 transposed shard,
            # computed as ones-vector matmuls on the otherwise-idle PE.
            # 4-way col-group packing (tile_position) runs 4 M=1 matmuls
            # concurrently in the array. at_f16 has A_hat baked in, so the
            # +1 self-loop is included. The last `nres` buffers stay resident
            # and are reused by P3 (which walks j blocks in reverse).
            deg_ps = pacc.tile([128, rows], f32, tag="bigacc")
            res_tiles = {}
            for it in range(nb // 2):
                if it >= nb // 2 - nres:
                    buf = p3res.tile(
                        [128, 2, rows], f16, tag="res", name=f"resbuf{it}"
                    )
                    res_tiles[it] = buf
                else:
                    buf = p3b.tile([128, 2, rows], f16, tag="bbuf")
                nc.sync.dma_start(
                    buf[:],
                    atb.ap()[it * 256 : (it + 1) * 256, :].rearrange(
                        "(t p) r -> p t r", p=128
                    ),
                )
                for t in range(2):
                    jb = 2 * it + t
                    k = jb % 4
                    for c in range(nch):
                        nc.tensor.matmul(
                            deg_ps[32 * k : 32 * k + 1, c * ch : (c + 1) * ch],
                            ones16[:],
                            buf[:, t, c * ch : (c + 1) * ch],
                            start=(jb == k),
                            stop=(jb == nb - 4 + k),
                            tile_position=(0, 32 * k),
                        )
            nc.vector.tensor_copy(deg_line[:], deg_ps[0:1, :])
            for k in range(1, 4):
                nc.vector.tensor_add(
                    deg_line[:], deg_line[:], deg_ps[32 * k : 32 * k + 1, :]
                )

            # AllGather degrees; local d read back from the DRAM bounce so no
            # core-dependent indexing is needed anywhere in the program
            nc.gpsimd.dma_start(
                cc_deg_in.ap().rearrange("(a b) -> a b", a=1), deg_line[:]
            )
            nc.scalar.dma_start(
                d_rc[:], cc_deg_in.ap().rearrange("(b p) -> p b", p=128)
            )
            nc.scalar.sqrt(d_rc[:], d_rc[:])
            nc.vector.reciprocal(d_rc[:], d_rc[:])
            nc.gpsimd.collective_compute(
                "AllGather",
                mybir.AluOpType.bypass,
                replica_groups=rgroups,
                ins=[cc_deg_in.ap()],
                outs=[cc_deg_out.ap()],
            )
            # chunked readback: one fat descriptor per chunk per partition
            dq = max(1, nb // 8)
            for g in range(nb // dq):
                nc.scalar.dma_start(
                    d_sb[:, g * dq : (g + 1) * dq],
                    cc_deg_out.ap()[g * dq * 128 : (g + 1) * dq * 128].rearrange(
                        "(b p) -> p b", p=128
                    ),
                )
            nc.scalar.sqrt(d_sb[:], d_sb[:])
            nc.vector.reciprocal(d_sb[:], d_sb[:])

            # X <- d * X (rows j scaled by d[j]); written as f32r for the PE.
            # Chunked so P2's first matmuls start before the whole scale ends.
            xq = max(1, nb // 4)
            for g in range(nb // xq):
                sl = slice(g * xq, (g + 1) * xq)
                nc.vector.tensor_tensor(
                    mm(x_sb[:, sl, :]),
                    x_sb[:, sl, :],
                    d_sb[:, sl].broadcast_to((128, xq, F)),
                    op=mybir.AluOpType.mult,
                )

            # ---- P2: AHXT accumulation over j blocks ----
            axt_ps = pacc.tile([128, rows], f32, tag="bigacc")
            for it in range(nb // 2):
                buf = strips.tile([128, 2, rows], f32, tag="abuf")
                nc.sync.dma_start(
                    mm(buf[:]),
                    at.ap()[it * 256 : (it + 1) * 256, :]
                    .rearrange("(t p) r -> p t r", p=128)
                    .bitcast(mmdt),
                )
                for t in range(2):
                    jb = 2 * it + t
                    for c in range(nch):
                        nc.tensor.matmul(
                            axt_ps[:, c * ch : (c + 1) * ch],
                            mm(x_sb[:, jb, :]),
                            mm(buf[:, t, c * ch : (c + 1) * ch]),
                            start=(jb == 0),
                            stop=(jb == nb - 1),
                        )
            for c in range(nch):
                nc.vector.tensor_copy(
                    axt_sb[:, c * ch : (c + 1) * ch], axt_ps[:, c * ch : (c + 1) * ch]
                )

            # ---- z, softmax(s), grouped in PSUM-bank-sized batches ----
            grp = min(4, rb)
            for g in range(rb // grp):
                lo, hi = g * grp, (g + 1) * grp
                z_ps = pmm.tile([128, grp, F], f32, tag="zps")
                sl_ps = pmm.tile([128, grp, F], f32, tag="zps")
                for k in range(grp):
                    ib = lo + k
                    nc.tensor.matmul(
                        z_ps[:, k, :], axt_sb[:, ib * 128 : (ib + 1) * 128], wet_sb[:]
                    )
                    nc.tensor.matmul(
                        sl_ps[:, k, :], axt_sb[:, ib * 128 : (ib + 1) * 128], wat_sb[:]
                    )
                d_bc = d_rc[:, lo:hi].broadcast_to((128, grp, F))
                nc.vector.tensor_tensor(
                    z_sb[:, lo:hi, :], z_ps[:], d_bc, op=mybir.AluOpType.mult
                )
                nc.vector.tensor_tensor(
                    sl_sb[:, lo:hi, :], sl_ps[:], d_bc, op=mybir.AluOpType.mult
                )
            bias_e = beb_sb[:].broadcast_to((128, F, rb)).rearrange("p f b -> p b f")
            bias_a = bab_sb[:].broadcast_to((128, F, rb)).rearrange("p f b -> p b f")
            nc.vector.tensor_tensor(z_sb[:], z_sb[:], bias_e, op=mybir.AluOpType.add)
            nc.vector.tensor_tensor(sl_sb[:], sl_sb[:], bias_a, op=mybir.AluOpType.add)
            nc.vector.reduce_max(
                mx[:], sl_sb[:], axis=mybir.AxisListType.X, negate=True
            )
            for ib in range(rb):
                nc.scalar.activation(
                    sl_sb[:, ib, :],
                    sl_sb[:, ib, :],
                    mybir.ActivationFunctionType.Exp,
                    bias=mx[:, ib, :],
                    accum_out=ssum[:, ib, :],
                )
            nc.vector.reciprocal(rsum[:], ssum[:])
            nc.vector.tensor_tensor(
                sl_sb[:],
                sl_sb[:],
                rsum[:].broadcast_to((128, rb, F)),
                op=mybir.AluOpType.mult,
            )

            # s shard out + AllGather (fp16 payload) + hoisted X_next partial
            xn_ps = pfin.tile([F, F], f32, tag="xnps")
            nc.gpsimd.dma_start(
                s_out.ap().rearrange("(b p) f -> p b f", p=128), sl_sb[:]
            )
            nc.vector.tensor_copy(s_bf[:], sl_sb[:])
            nc.gpsimd.dma_start(
                cc_s_in.ap().rearrange("(b p) f -> p b f", p=128), s_bf[:]
            )
            nc.gpsimd.collective_compute(
                "AllGather",
                mybir.AluOpType.bypass,
                replica_groups=rgroups,
                ins=[cc_s_in.ap()],
                outs=[cc_s_out.ap()],
            )
            for ib in range(rb):
                nc.tensor.matmul(
                    xn_ps[:],
                    sl_sb[:, ib, :],
                    z_sb[:, ib, :],
                    start=(ib == 0),
                    stop=(ib == rb - 1),
                )

            # ---- P3: AhatS_T accumulation over j blocks ----
            # Reverse j order so the first `nres` buffers are P1's residents:
            # P3's matmuls start right after the AllGather with no strip DMA.
            ast_ps = pacc.tile([128, rows], f32, tag="bigacc")
            order = list(reversed(range(nb // 2)))
            for pos, it in enumerate(order):
                if it in res_tiles:
                    buf = res_tiles[it]
                else:
                    buf = p3b.tile([128, 2, rows], f16, tag="bbuf")
                    nc.sync.dma_start(
                        buf[:],
                        atb.ap()[it * 256 : (it + 1) * 256, :].rearrange(
                            "(t p) r -> p t r", p=128
                        ),
                    )
                for t in range(2):
                    jb = 2 * it + t
                    sf = sfp.tile([128, F], f16, tag="sfull")
                    nc.scalar.dma_start(
                        sf[:], cc_s_out.ap()[jb * 128 : (jb + 1) * 128, :]
                    )
                    for c in range(nch):
                        nc.tensor.matmul(
                            ast_ps[:, c * ch : (c + 1) * ch],
                            sf[:],
                            buf[:, t, c * ch : (c + 1) * ch],
                            start=(pos == 0 and t == 0),
                            stop=(pos == nb // 2 - 1 and t == 1),
                        )
            for c in range(nch):
                nc.vector.tensor_copy(
                    ast_sb[:, c * ch : (c + 1) * ch], ast_ps[:, c * ch : (c + 1) * ch]
                )

            # As = AhatS_T.T - s_loc ; then final contraction
            an_ps = pfin.tile([F, F], f32, tag="anps")
            for ib in range(rb):
                tr_ps = pmm.tile([128, 128], f32, tag="zps")
                nc.tensor.transpose(
                    tr_ps[:], ast_sb[:, ib * 128 : (ib + 1) * 128], ident[:]
                )
                nc.vector.tensor_sub(as_sb[:, ib, :], tr_ps[:], sl_sb[:, ib, :])
            for ib in range(rb):
                nc.tensor.matmul(
                    an_ps[:],
                    sl_sb[:, ib, :],
                    as_sb[:, ib, :],
                    start=(ib == 0),
                    stop=(ib == rb - 1),
                )
            nc.vector.tensor_copy(xn_sb[:], xn_ps[:])
            nc.vector.tensor_copy(an_sb[:], an_ps[:])
            nc.gpsimd.dma_start(xn_out.ap(), xn_sb[:])
            nc.gpsimd.dma_start(an_out.ap(), an_sb[:])

    nc.compile()
    return nc


def make_in_maps(X, A, W_embed, b_embed, W_assign, b_assign, n_cores=CORES_FULL):
    """Host-side sharding / layout prep. Returns one input dict per core."""
    X = np.ascontiguousarray(np.asarray(X, dtype=np.float32))
    A = np.asarray(A, dtype=np.float32)
    n = A.shape[0]
    rows = n // n_cores
    we_t = np.ascontiguousarray(np.asarray(W_embed, np.float32).T)
    wa_t = np.ascontiguousarray(np.asarray(W_assign, np.float32).T)
    be_b = np.ascontiguousarray(
        np.broadcast_to(np.asarray(b_embed, np.float32)[None, :], (128, F))
    )
    ba_b = np.ascontiguousarray(
        np.broadcast_to(np.asarray(b_assign, np.float32)[None, :], (128, F))
    )
    in_maps = []
    for c in range(n_cores):
        at = np.ascontiguousarray(A[c * rows : (c + 1) * rows, :].T)
        # bake A_hat = A + I into the shard
        at[np.arange(c * rows, (c + 1) * rows), np.arange(rows)] += 1.0
        in_maps.append(
            {
                "at_hat": at,
                "at_f16": at.astype(np.float16),
                "x_in": X,
                "we_t": we_t,
                "wa_t": wa_t,
                "be_b": be_b,
                "ba_b": ba_b,
            }
        )
    return in_maps


_CACHE = {}


def _get_program(n, n_cores):
    key = (n, n_cores)
    if key not in _CACHE:
        _CACHE[key] = build_program(n, n_cores)
    return _CACHE[key]


def run_on_hw(inputs, n_cores=CORES_FULL, trace=False):
    """Compile (cached), run on hardware, return (outputs_tuple, BassKernelResults)."""
    from concourse.bass_interp import get_hw_module

    n = inputs["A"].shape[0]
    nc = _get_program(n, n_cores)
    in_maps = make_in_maps(n_cores=n_cores, **inputs)
    old_m = nc.m
    nc.m = get_hw_module(nc.m)
    try:
        res = run_bass_kernel_spmd(
            nc, in_maps, core_ids=list(range(n_cores)), trace=trace
        )
    finally:
        nc.m = old_m
    outs = res.results
    x_next = np.sum([r["xn_part"] for r in outs], axis=0, dtype=np.float32)
    a_next = np.sum([r["an_part"] for r in outs], axis=0, dtype=np.float32)
    s_l = np.concatenate([r["s_part"] for r in outs], axis=0)
    return (x_next, a_next, s_l), res


def kernel(X, A, W_embed, b_embed, W_assign, b_assign):
    out, _ = run_on_hw(
        dict(
            X=X,
            A=A,
            W_embed=W_embed,
            b_embed=b_embed,
            W_assign=W_assign,
            b_assign=b_assign,
        )
    )
    return out


# revision 50
# speedup vs baseline: 1.4640x; 1.0469x over previous
"""DiffPool forward (GCN embed/assign + pooled X/A) on 8 trn2 NeuronCores.

Sharding: 1D node partition. Core c owns rows R_c = [c*ROWS, (c+1)*ROWS).
Host feeds each core A_T_hat_c = (A[R_c,:] + I[R_c,:]).T  -> [N, ROWS], so the
contraction index j sits on the SBUF partition axis for both big products and
no transposes of A are ever done on device.

Device program (single SPMD NEFF, collectives across 8 cores):
  P1: deg_loc = column-sums of fp16 A_T_hat strips via ones-vector matmuls on
      the otherwise-idle PE (4-way tile_position col packing). The tail strip
      buffers stay resident in SBUF for P3 to reuse.
      AllGather(deg) -> d = 1/sqrt(deg)
  P2: AHXT[f,i]   = sum_j (d*X)[j,f] * A_T_hat[j,i]   (fp32r PSUM accum, f32 strips)
      z   = d_i * (AHXT.T @ We.T) + be
      s   = softmax(d_i * (AHXT.T @ Wa.T) + ba)       (row softmax, free axis)
      AllGather(s in fp16) -> s_full
  P3: AhatS_T[m,i] = sum_j s_full[j,m] * A_T_hat[j,i] (fp16 strips, reversed j
      order so P1's resident buffers are hit first)
      As = AhatS_T.T - s_loc                          (undo baked +I)
      Xn_part = s_loc.T @ z ; An_part = s_loc.T @ As  (partials over local rows)
Host: X_next = sum_c Xn_part, A_next = sum_c An_part, s_l = concat_c(s_part).

Precision: P2 (mixed-sign contraction) keeps fp32-width data (fp32r matmul
mode, ~1e-5 rel err). deg and A@s are nonnegative sums of ~8192 terms, so
independent fp16 input-rounding attenuates by ~1/sqrt(N): fp16 there costs
~1e-5..1e-4 rel err on A_next only, and halves those passes' DMA.
"""

import numpy as np

import concourse.bass as bass
import concourse.mybir as mybir
import concourse.tile as tile
from concourse import bacc
from concourse.bass_utils import run_bass_kernel_spmd
from concourse.masks import make_identity

F = 128  # F_IN == N_META == N_EMBED == 128
f32 = mybir.dt.float32
N_FULL = 8192
CORES_FULL = 8


def build_program(n=N_FULL, n_cores=CORES_FULL, fast32=True):
    """Build the SPMD single-core program (same NEFF on all cores)."""
    rows = n // n_cores      # local node rows per core
    nb = n // 128            # j blocks (global)
    rb = rows // 128         # local row blocks
    ch = min(512, rows)      # matmul moving-dim chunk
    nch = rows // ch         # chunks per strip
    assert nb % 2 == 0 and rows % 128 == 0
    mmdt = mybir.dt.float32r if fast32 else f32

    def mm(ap):
        return ap.bitcast(mmdt)

    nc = bacc.Bacc(
        "TRN2",
        target_bir_lowering=False,
        debug=False,
        enable_asserts=False,
        num_devices=n_cores,
    )

    f16 = mybir.dt.float16
    at = nc.dram_tensor("at_hat", [n, rows], f32, kind="ExternalInput")
    atb = nc.dram_tensor("at_f16", [n, rows], f16, kind="ExternalInput")
    x_in = nc.dram_tensor("x_in", [n, F], f32, kind="ExternalInput")
    wet = nc.dram_tensor("we_t", [F, F], f32, kind="ExternalInput")
    wat = nc.dram_tensor("wa_t", [F, F], f32, kind="ExternalInput")
    beb = nc.dram_tensor("be_b", [128, F], f32, kind="ExternalInput")
    bab = nc.dram_tensor("ba_b", [128, F], f32, kind="ExternalInput")

    s_out = nc.dram_tensor("s_part", [rows, F], f32, kind="ExternalOutput")
    xn_out = nc.dram_tensor("xn_part", [F, F], f32, kind="ExternalOutput")
    an_out = nc.dram_tensor("an_part", [F, F], f32, kind="ExternalOutput")

    cc_deg_in = nc.dram_tensor("cc_deg_in", [rows], f32, kind="Internal")
    cc_deg_out = nc.dram_tensor(
        "cc_deg_out", [n], f32, kind="Internal", addr_space="Shared"
    )
    cc_s_in = nc.dram_tensor("cc_s_in", [rows, F], f16, kind="Internal")
    cc_s_out = nc.dram_tensor(
        "cc_s_out", [n, F], f16, kind="Internal", addr_space="Shared"
    )
    rgroups = [list(range(n_cores))]

    nres = min(8, nb // 2)   # P1 tail buffers kept resident for P3 reuse

    with tile.TileContext(nc) as tc:
        with (
            tc.tile_pool(name="strips", bufs=10) as strips,
            tc.tile_pool(name="p3b", bufs=5) as p3b,
            tc.tile_pool(name="p3res", bufs=nres) as p3res,
            tc.tile_pool(name="persist", bufs=1) as persist,
            tc.tile_pool(name="sf", bufs=4) as sfp,
            tc.tile_pool(name="pacc", bufs=1, space="PSUM") as pacc,
            tc.tile_pool(name="pmm", bufs=2, space="PSUM") as pmm,
            tc.tile_pool(name="pfin", bufs=1, space="PSUM") as pfin,
        ):
            # ---- persistent sbuf tensors ----
            x_sb = persist.tile([128, nb, F], f32)      # X, later d*X (j on partitions)
            ones16 = persist.tile([128, 1], f16)
            deg_line = persist.tile([1, rows], f32)
            d_sb = persist.tile([128, nb], f32)         # 1/sqrt(deg) full, [p, blk]
            d_rc = persist.tile([128, rb], f32)         # same for local rows
            wet_sb = persist.tile([F, F], f32)
            wat_sb = persist.tile([F, F], f32)
            beb_sb = persist.tile([128, F], f32)
            bab_sb = persist.tile([128, F], f32)
            ident = persist.tile([128, 128], f32)
            axt_sb = persist.tile([128, rows], f32)     # AHXT
            z_sb = persist.tile([128, rb, F], f32)
            sl_sb = persist.tile([128, rb, F], f32)     # logits -> s
            s_bf = persist.tile([128, rb, F], f16)      # s in fp16 for P3
            as_sb = persist.tile([128, rb, F], f32)     # A @ s (local rows)
            ast_sb = persist.tile([128, rows], f32)     # AhatS_T
            mx = persist.tile([128, rb, 1], f32)
            ssum = persist.tile([128, rb, 1], f32)
            rsum = persist.tile([128, rb, 1], f32)
            xn_sb = persist.tile([F, F], f32)
            an_sb = persist.tile([F, F], f32)

            # ---- constant loads / setup ----
            # nc.sync is reserved for the big strip streams; small loads go on
            # gpsimd/scalar so a dependent DMA never blocks strip prefetch.
            nc.sync.dma_start(
                mm(x_sb[:]), x_in.ap().rearrange("(b p) f -> p b f", p=128).bitcast(mmdt)
            )
            nc.gpsimd.dma_start(wet_sb[:], wet.ap())
            nc.gpsimd.dma_start(wat_sb[:], wat.ap())
            nc.gpsimd.dma_start(beb_sb[:], beb.ap())
            nc.gpsimd.dma_start(bab_sb[:], bab.ap())
            make_identity(nc, ident[:])
            nc.vector.memset(ones16[:], 1.0)

            # ---- P1: degrees = column sums of the fp16 transposed shard,
            # computed as ones-vector matmuls on the otherwise-idle PE.
            # 4-way col-group packing (tile_position) runs 4 M=1 matmuls
            # concurrently in the array. at_f16 has A_hat baked in, so the
            # +1 self-loop is included. The last `nres` buffers stay resident
            # and are reused by P3 (which walks j blocks in reverse).
            deg_ps = pacc.tile([128, rows], f32, tag="bigacc")
            res_tiles = {}
            for it in range(nb // 2):
                if it >= nb // 2 - nres:
                    buf = p3res.tile(
                        [128, 2, rows], f16, tag="res", name=f"resbuf{it}"
                    )
                    res_tiles[it] = buf
                else:
                    buf = p3b.tile([128, 2, rows], f16, tag="bbuf")
                nc.sync.dma_start(
                    buf[:],
                    atb.ap()[it * 256 : (it + 1) * 256, :].rearrange(
                        "(t p) r -> p t r", p=128
                    ),
                )
                for t in range(2):
                    jb = 2 * it + t
                    k = jb % 4
                    for c in range(nch):
                        nc.tensor.matmul(
                            deg_ps[32 * k : 32 * k + 1, c * ch : (c + 1) * ch],
                            ones16[:],
                            buf[:, t, c * ch : (c + 1) * ch],
                            start=(jb == k),
                            stop=(jb == nb - 4 + k),
                            tile_position=(0, 32 * k),
                        )
            nc.vector.tensor_copy(deg_line[:], deg_ps[0:1, :])
            for k in range(1, 4):
                nc.vector.tensor_add(
                    deg_line[:], deg_line[:], deg_ps[32 * k : 32 * k + 1, :]
                )

            # AllGather degrees; local d read back from the DRAM bounce so no
            # core-dependent indexing is needed anywhere in the program
            nc.gpsimd.dma_start(
                cc_deg_in.ap().rearrange("(a b) -> a b", a=1), deg_line[:]
            )
            nc.scalar.dma_start(
                d_rc[:], cc_deg_in.ap().rearrange("(b p) -> p b", p=128)
            )
            nc.scalar.sqrt(d_rc[:], d_rc[:])
            nc.vector.reciprocal(d_rc[:], d_rc[:])
            nc.gpsimd.collective_compute(
                "AllGather",
                mybir.AluOpType.bypass,
                replica_groups=rgroups,
                ins=[cc_deg_in.ap()],
                outs=[cc_deg_out.ap()],
            )
            # chunked d readback + rsqrt + X scale, pipelined so P2's first
            # matmuls start as soon as chunk 0 is ready
            dq = max(1, nb // 8)
            for g in range(nb // dq):
                sl = slice(g * dq, (g + 1) * dq)
                nc.scalar.dma_start(
                    d_sb[:, sl],
                    cc_deg_out.ap()[g * dq * 128 : (g + 1) * dq * 128].rearrange(
                        "(b p) -> p b", p=128
                    ),
                )
                nc.scalar.sqrt(d_sb[:, sl], d_sb[:, sl])
                nc.vector.reciprocal(d_sb[:, sl], d_sb[:, sl])
                nc.vector.tensor_tensor(
                    mm(x_sb[:, sl, :]),
                    x_sb[:, sl, :],
                    d_sb[:, sl].broadcast_to((128, dq, F)),
                    op=mybir.AluOpType.mult,
                )

            # ---- P2: AHXT accumulation over j blocks ----
            axt_ps = pacc.tile([128, rows], f32, tag="bigacc")
            for it in range(nb // 2):
                buf = strips.tile([128, 2, rows], f32, tag="abuf")
                nc.sync.dma_start(
                    mm(buf[:]),
                    at.ap()[it * 256 : (it + 1) * 256, :]
                    .rearrange("(t p) r -> p t r", p=128)
                    .bitcast(mmdt),
                )
                for t in range(2):
                    jb = 2 * it + t
                    for c in range(nch):
                        nc.tensor.matmul(
                            axt_ps[:, c * ch : (c + 1) * ch],
                            mm(x_sb[:, jb, :]),
                            mm(buf[:, t, c * ch : (c + 1) * ch]),
                            start=(jb == 0),
                            stop=(jb == nb - 1),
                        )
            for c in range(nch):
                nc.vector.tensor_copy(
                    axt_sb[:, c * ch : (c + 1) * ch], axt_ps[:, c * ch : (c + 1) * ch]
                )

            # ---- z, softmax(s), grouped in PSUM-bank-sized batches ----
            grp = min(4, rb)
            for g in range(rb // grp):
                lo, hi = g * grp, (g + 1) * grp
                z_ps = pmm.tile([128, grp, F], f32, tag="zps")
                sl_ps = pmm.tile([128, grp, F], f32, tag="zps")
                for k in range(grp):
                    ib = lo + k
                    nc.tensor.matmul(
                        z_ps[:, k, :], axt_sb[:, ib * 128 : (ib + 1) * 128], wet_sb[:]
                    )
                    nc.tensor.matmul(
                        sl_ps[:, k, :], axt_sb[:, ib * 128 : (ib + 1) * 128], wat_sb[:]
                    )
                d_bc = d_rc[:, lo:hi].broadcast_to((128, grp, F))
                nc.vector.tensor_tensor(
                    z_sb[:, lo:hi, :], z_ps[:], d_bc, op=mybir.AluOpType.mult
                )
                nc.vector.tensor_tensor(
                    sl_sb[:, lo:hi, :], sl_ps[:], d_bc, op=mybir.AluOpType.mult
                )
            bias_e = beb_sb[:].broadcast_to((128, F, rb)).rearrange("p f b -> p b f")
            bias_a = bab_sb[:].broadcast_to((128, F, rb)).rearrange("p f b -> p b f")
            nc.vector.tensor_tensor(z_sb[:], z_sb[:], bias_e, op=mybir.AluOpType.add)
            nc.vector.tensor_tensor(sl_sb[:], sl_sb[:], bias_a, op=mybir.AluOpType.add)
            nc.vector.reduce_max(
                mx[:], sl_sb[:], axis=mybir.AxisListType.X, negate=True
            )
            for ib in range(rb):
                nc.scalar.activation(
                    sl_sb[:, ib, :],
                    sl_sb[:, ib, :],
                    mybir.ActivationFunctionType.Exp,
                    bias=mx[:, ib, :],
                    accum_out=ssum[:, ib, :],
                )
            nc.vector.reciprocal(rsum[:], ssum[:])
            nc.vector.tensor_tensor(
                sl_sb[:],
                sl_sb[:],
                rsum[:].broadcast_to((128, rb, F)),
                op=mybir.AluOpType.mult,
            )

            # s shard out + AllGather (fp16 payload) + hoisted X_next partial
            xn_ps = pfin.tile([F, F], f32, tag="xnps")
            nc.gpsimd.dma_start(
                s_out.ap().rearrange("(b p) f -> p b f", p=128), sl_sb[:]
            )
            nc.vector.tensor_copy(s_bf[:], sl_sb[:])
            nc.gpsimd.dma_start(
                cc_s_in.ap().rearrange("(b p) f -> p b f", p=128), s_bf[:]
            )
            nc.gpsimd.collective_compute(
                "AllGather",
                mybir.AluOpType.bypass,
                replica_groups=rgroups,
                ins=[cc_s_in.ap()],
                outs=[cc_s_out.ap()],
            )
            for ib in range(rb):
                nc.tensor.matmul(
                    xn_ps[:],
                    sl_sb[:, ib, :],
                    z_sb[:, ib, :],
                    start=(ib == 0),
                    stop=(ib == rb - 1),
                )

            # ---- P3: AhatS_T accumulation over j blocks ----
            # Reverse j order so the first `nres` buffers are P1's residents:
            # P3's matmuls start right after the AllGather with no strip DMA.
            ast_ps = pacc.tile([128, rows], f32, tag="bigacc")
            order = list(reversed(range(nb // 2)))
            for pos, it in enumerate(order):
                if it in res_tiles:
                    buf = res_tiles[it]
                else:
                    buf = p3b.tile([128, 2, rows], f16, tag="bbuf")
                    nc.sync.dma_start(
                        buf[:],
                        atb.ap()[it * 256 : (it + 1) * 256, :].rearrange(
                            "(t p) r -> p t r", p=128
                        ),
                    )
                for t in range(2):
                    jb = 2 * it + t
                    sf = sfp.tile([128, F], f16, tag="sfull")
                    nc.scalar.dma_start(
                        sf[:], cc_s_out.ap()[jb * 128 : (jb + 1) * 128, :]
                    )
                    for c in range(nch):
                        nc.tensor.matmul(
                            ast_ps[:, c * ch : (c + 1) * ch],
                            sf[:],
                            buf[:, t, c * ch : (c + 1) * ch],
                            start=(pos == 0 and t == 0),
                            stop=(pos == nb // 2 - 1 and t == 1),
                        )
            for c in range(nch):
                nc.vector.tensor_copy(
                    ast_sb[:, c * ch : (c + 1) * ch], ast_ps[:, c * ch : (c + 1) * ch]
                )

            # As = AhatS_T.T - s_loc ; then final contraction
            an_ps = pfin.tile([F, F], f32, tag="anps")
            for ib in range(rb):
                tr_ps = pmm.tile([128, 128], f32, tag="zps")
                nc.tensor.transpose(
                    tr_ps[:], ast_sb[:, ib * 128 : (ib + 1) * 128], ident[:]
                )
                nc.vector.tensor_sub(as_sb[:, ib, :], tr_ps[:], sl_sb[:, ib, :])
            for ib in range(rb):
                nc.tensor.matmul(
                    an_ps[:],
                    sl_sb[:, ib, :],
                    as_sb[:, ib, :],
                    start=(ib == 0),
                    stop=(ib == rb - 1),
                )
            nc.vector.tensor_copy(xn_sb[:], xn_ps[:])
            nc.vector.tensor_copy(an_sb[:], an_ps[:])
            nc.gpsimd.dma_start(xn_out.ap(), xn_sb[:])
            nc.gpsimd.dma_start(an_out.ap(), an_sb[:])

    nc.compile()
    return nc


def make_in_maps(X, A, W_embed, b_embed, W_assign, b_assign, n_cores=CORES_FULL):
    """Host-side sharding / layout prep. Returns one input dict per core."""
    X = np.ascontiguousarray(np.asarray(X, dtype=np.float32))
    A = np.asarray(A, dtype=np.float32)
    n = A.shape[0]
    rows = n // n_cores
    we_t = np.ascontiguousarray(np.asarray(W_embed, np.float32).T)
    wa_t = np.ascontiguousarray(np.asarray(W_assign, np.float32).T)
    be_b = np.ascontiguousarray(
        np.broadcast_to(np.asarray(b_embed, np.float32)[None, :], (128, F))
    )
    ba_b = np.ascontiguousarray(
        np.broadcast_to(np.asarray(b_assign, np.float32)[None, :], (128, F))
    )
    in_maps = []
    for c in range(n_cores):
        at = np.ascontiguousarray(A[c * rows : (c + 1) * rows, :].T)
        # bake A_hat = A + I into the shard
        at[np.arange(c * rows, (c + 1) * rows), np.arange(rows)] += 1.0
        in_maps.append(
            {
                "at_hat": at,
                "at_f16": at.astype(np.float16),
                "x_in": X,
                "we_t": we_t,
                "wa_t": wa_t,
                "be_b": be_b,
                "ba_b": ba_b,
            }
        )
    return in_maps


_CACHE = {}


def _get_program(n, n_cores):
    key = (n, n_cores)
    if key not in _CACHE:
        _CACHE[key] = build_program(n, n_cores)
    return _CACHE[key]


def run_on_hw(inputs, n_cores=CORES_FULL, trace=False):
    """Compile (cached), run on hardware, return (outputs_tuple, BassKernelResults)."""
    from concourse.bass_interp import get_hw_module

    n = inputs["A"].shape[0]
    nc = _get_program(n, n_cores)
    in_maps = make_in_maps(n_cores=n_cores, **inputs)
    old_m = nc.m
    nc.m = get_hw_module(nc.m)
    try:
        res = run_bass_kernel_spmd(
            nc, in_maps, core_ids=list(range(n_cores)), trace=trace
        )
    finally:
        nc.m = old_m
    outs = res.results
    x_next = np.sum([r["xn_part"] for r in outs], axis=0, dtype=np.float32)
    a_next = np.sum([r["an_part"] for r in outs], axis=0, dtype=np.float32)
    s_l = np.concatenate([r["s_part"] for r in outs], axis=0)
    return (x_next, a_next, s_l), res


def kernel(X, A, W_embed, b_embed, W_assign, b_assign):
    out, _ = run_on_hw(
        dict(
            X=X,
            A=A,
            W_embed=W_embed,
            b_embed=b_embed,
            W_assign=W_assign,
            b_assign=b_assign,
        )
    )
    return out


# revision 52
# speedup vs baseline: 1.4955x; 1.0215x over previous
"""DiffPool forward (GCN embed/assign + pooled X/A) on 8 trn2 NeuronCores.

Sharding: 1D node partition. Core c owns rows R_c = [c*ROWS, (c+1)*ROWS).
Host feeds each core A_T_hat_c = (A[R_c,:] + I[R_c,:]).T  -> [N, ROWS], so the
contraction index j sits on the SBUF partition axis for both big products and
no transposes of A are ever done on device.

Device program (single SPMD NEFF, collectives across 8 cores):
  P1: deg_loc = column-sums of fp16 A_T_hat strips via ones-vector matmuls on
      the otherwise-idle PE (4-way tile_position col packing). The tail strip
      buffers stay resident in SBUF for P3 to reuse.
      AllGather(deg) -> d = 1/sqrt(deg)
  P2: AHXT[f,i]   = sum_j (d*X)[j,f] * A_T_hat[j,i]   (fp32r PSUM accum, f32 strips)
      z   = d_i * (AHXT.T @ We.T) + be
      s   = softmax(d_i * (AHXT.T @ Wa.T) + ba)       (row softmax, free axis)
      AllGather(s in fp16) -> s_full
  P3: AhatS_T[m,i] = sum_j s_full[j,m] * A_T_hat[j,i] (fp16 strips, reversed j
      order so P1's resident buffers are hit first)
      As = AhatS_T.T - s_loc                          (undo baked +I)
      Xn_part = s_loc.T @ z ; An_part = s_loc.T @ As  (partials over local rows)
Host: X_next = sum_c Xn_part, A_next = sum_c An_part, s_l = concat_c(s_part).

Precision: P2 (mixed-sign contraction) keeps fp32-width data (fp32r matmul
mode, ~1e-5 rel err). deg and A@s are nonnegative sums of ~8192 terms, so
independent fp16 input-rounding attenuates by ~1/sqrt(N): fp16 there costs
~1e-5..1e-4 rel err on A_next only, and halves those passes' DMA.
"""

import numpy as np

import concourse.bass as bass
import concourse.mybir as mybir
import concourse.tile as tile
from concourse import bacc
from concourse.bass_utils import run_bass_kernel_spmd
from concourse.masks import make_identity

F = 128  # F_IN == N_META == N_EMBED == 128
f32 = mybir.dt.float32
N_FULL = 8192
CORES_FULL = 8


def build_program(n=N_FULL, n_cores=CORES_FULL, fast32=True):
    """Build the SPMD single-core program (same NEFF on all cores)."""
    rows = n // n_cores      # local node rows per core
    nb = n // 128            # j blocks (global)
    rb = rows // 128         # local row blocks
    ch = min(512, rows)      # matmul moving-dim chunk
    nch = rows // ch         # chunks per strip
    assert nb % 2 == 0 and rows % 128 == 0
    mmdt = mybir.dt.float32r if fast32 else f32

    def mm(ap):
        return ap.bitcast(mmdt)

    nc = bacc.Bacc(
        "TRN2",
        target_bir_lowering=False,
        debug=False,
        enable_asserts=False,
        num_devices=n_cores,
    )

    f16 = mybir.dt.float16
    at = nc.dram_tensor("at_hat", [n, rows], f32, kind="ExternalInput")
    atb = nc.dram_tensor("at_f16", [n, rows], f16, kind="ExternalInput")
    x_in = nc.dram_tensor("x_in", [n, F], f32, kind="ExternalInput")
    wet = nc.dram_tensor("we_t", [F, F], f32, kind="ExternalInput")
    wat = nc.dram_tensor("wa_t", [F, F], f32, kind="ExternalInput")
    beb = nc.dram_tensor("be_b", [128, F], f32, kind="ExternalInput")
    bab = nc.dram_tensor("ba_b", [128, F], f32, kind="ExternalInput")

    s_out = nc.dram_tensor("s_part", [rows, F], f32, kind="ExternalOutput")
    xn_out = nc.dram_tensor("xn_part", [F, F], f32, kind="ExternalOutput")
    an_out = nc.dram_tensor("an_part", [F, F], f32, kind="ExternalOutput")

    cc_warm_in = nc.dram_tensor("cc_warm_in", [8], f32, kind="Internal")
    cc_warm_out = nc.dram_tensor(
        "cc_warm_out", [8 * n_cores], f32, kind="Internal", addr_space="Shared"
    )
    cc_deg_in = nc.dram_tensor("cc_deg_in", [rows], f32, kind="Internal")
    cc_deg_out = nc.dram_tensor(
        "cc_deg_out", [n], f32, kind="Internal", addr_space="Shared"
    )
    cc_s_in = nc.dram_tensor("cc_s_in", [rows, F], f16, kind="Internal")
    cc_s_out = nc.dram_tensor(
        "cc_s_out", [n, F], f16, kind="Internal", addr_space="Shared"
    )
    rgroups = [list(range(n_cores))]

    nres = min(8, nb // 2)   # P1 tail buffers kept resident for P3 reuse

    with tile.TileContext(nc) as tc:
        with (
            tc.tile_pool(name="strips", bufs=10) as strips,
            tc.tile_pool(name="p3b", bufs=5) as p3b,
            tc.tile_pool(name="p3res", bufs=nres) as p3res,
            tc.tile_pool(name="persist", bufs=1) as persist,
            tc.tile_pool(name="sf", bufs=4) as sfp,
            tc.tile_pool(name="pacc", bufs=1, space="PSUM") as pacc,
            tc.tile_pool(name="pmm", bufs=2, space="PSUM") as pmm,
            tc.tile_pool(name="pfin", bufs=1, space="PSUM") as pfin,
        ):
            # ---- persistent sbuf tensors ----
            x_sb = persist.tile([128, nb, F], f32)      # X, later d*X (j on partitions)
            ones16 = persist.tile([128, 1], f16)
            deg_line = persist.tile([1, rows], f32)
            d_sb = persist.tile([128, nb], f32)         # 1/sqrt(deg) full, [p, blk]
            d_rc = persist.tile([128, rb], f32)         # same for local rows
            wet_sb = persist.tile([F, F], f32)
            wat_sb = persist.tile([F, F], f32)
            beb_sb = persist.tile([128, F], f32)
            bab_sb = persist.tile([128, F], f32)
            ident = persist.tile([128, 128], f32)
            axt_sb = persist.tile([128, rows], f32)     # AHXT
            z_sb = persist.tile([128, rb, F], f32)
            sl_sb = persist.tile([128, rb, F], f32)     # logits -> s
            s_bf = persist.tile([128, rb, F], f16)      # s in fp16 for P3
            as_sb = persist.tile([128, rb, F], f32)     # A @ s (local rows)
            ast_sb = persist.tile([128, rows], f32)     # AhatS_T
            mx = persist.tile([128, rb, 1], f32)
            ssum = persist.tile([128, rb, 1], f32)
            rsum = persist.tile([128, rb, 1], f32)
            xn_sb = persist.tile([F, F], f32)
            an_sb = persist.tile([F, F], f32)

            # ---- constant loads / setup ----
            # nc.sync is reserved for the big strip streams; small loads go on
            # gpsimd/scalar so a dependent DMA never blocks strip prefetch.
            nc.sync.dma_start(
                mm(x_sb[:]), x_in.ap().rearrange("(b p) f -> p b f", p=128).bitcast(mmdt)
            )
            nc.gpsimd.dma_start(wet_sb[:], wet.ap())
            nc.gpsimd.dma_start(wat_sb[:], wat.ap())
            nc.gpsimd.dma_start(beb_sb[:], beb.ap())
            nc.gpsimd.dma_start(bab_sb[:], bab.ap())
            make_identity(nc, ident[:])
            nc.vector.memset(ones16[:], 1.0)

            # warmup collective: absorbs ncfw first-collective startup cost
            # concurrently with P1's DMA stream
            warm_sb = persist.tile([1, 8 * n_cores], f32)
            nc.vector.memset(warm_sb[:], 0.0)
            nc.gpsimd.dma_start(
                cc_warm_in.ap().rearrange("(a b) -> a b", a=1), warm_sb[:, 0:8]
            )
            nc.gpsimd.collective_compute(
                "AllGather",
                mybir.AluOpType.bypass,
                replica_groups=rgroups,
                ins=[cc_warm_in.ap()],
                outs=[cc_warm_out.ap()],
            )
            nc.gpsimd.dma_start(
                warm_sb[:], cc_warm_out.ap().rearrange("(a b) -> a b", a=1)
            )

            # ---- P1: degrees = column sums of the fp16 transposed shard,
            # computed as ones-vector matmuls on the otherwise-idle PE.
            # 4-way col-group packing (tile_position) runs 4 M=1 matmuls
            # concurrently in the array. at_f16 has A_hat baked in, so the
            # +1 self-loop is included. The last `nres` buffers stay resident
            # and are reused by P3 (which walks j blocks in reverse).
            deg_ps = pacc.tile([128, rows], f32, tag="bigacc")
            res_tiles = {}
            for it in range(nb // 2):
                if it >= nb // 2 - nres:
                    buf = p3res.tile(
                        [128, 2, rows], f16, tag="res", name=f"resbuf{it}"
                    )
                    res_tiles[it] = buf
                else:
                    buf = p3b.tile([128, 2, rows], f16, tag="bbuf")
                nc.sync.dma_start(
                    buf[:],
                    atb.ap()[it * 256 : (it + 1) * 256, :].rearrange(
                        "(t p) r -> p t r", p=128
                    ),
                )
                for t in range(2):
                    jb = 2 * it + t
                    k = jb % 4
                    for c in range(nch):
                        nc.tensor.matmul(
                            deg_ps[32 * k : 32 * k + 1, c * ch : (c + 1) * ch],
                            ones16[:],
                            buf[:, t, c * ch : (c + 1) * ch],
                            start=(jb == k),
                            stop=(jb == nb - 4 + k),
                            tile_position=(0, 32 * k),
                        )
            nc.vector.tensor_copy(deg_line[:], deg_ps[0:1, :])
            for k in range(1, 4):
                nc.vector.tensor_add(
                    deg_line[:], deg_line[:], deg_ps[32 * k : 32 * k + 1, :]
                )

            # AllGather degrees; local d read back from the DRAM bounce so no
            # core-dependent indexing is needed anywhere in the program
            nc.gpsimd.dma_start(
                cc_deg_in.ap().rearrange("(a b) -> a b", a=1), deg_line[:]
            )
            nc.scalar.dma_start(
                d_rc[:], cc_deg_in.ap().rearrange("(b p) -> p b", p=128)
            )
            nc.scalar.sqrt(d_rc[:], d_rc[:])
            nc.vector.reciprocal(d_rc[:], d_rc[:])
            nc.gpsimd.collective_compute(
                "AllGather",
                mybir.AluOpType.bypass,
                replica_groups=rgroups,
                ins=[cc_deg_in.ap()],
                outs=[cc_deg_out.ap()],
            )
            # chunked d readback + rsqrt + X scale, pipelined so P2's first
            # matmuls start as soon as chunk 0 is ready
            dq = max(1, nb // 8)
            for g in range(nb // dq):
                sl = slice(g * dq, (g + 1) * dq)
                nc.scalar.dma_start(
                    d_sb[:, sl],
                    cc_deg_out.ap()[g * dq * 128 : (g + 1) * dq * 128].rearrange(
                        "(b p) -> p b", p=128
                    ),
                )
                nc.scalar.sqrt(d_sb[:, sl], d_sb[:, sl])
                nc.vector.reciprocal(d_sb[:, sl], d_sb[:, sl])
                nc.vector.tensor_tensor(
                    mm(x_sb[:, sl, :]),
                    x_sb[:, sl, :],
                    d_sb[:, sl].broadcast_to((128, dq, F)),
                    op=mybir.AluOpType.mult,
                )

            # ---- P2: AHXT accumulation over j blocks ----
            axt_ps = pacc.tile([128, rows], f32, tag="bigacc")
            for it in range(nb // 2):
                buf = strips.tile([128, 2, rows], f32, tag="abuf")
                nc.sync.dma_start(
                    mm(buf[:]),
                    at.ap()[it * 256 : (it + 1) * 256, :]
                    .rearrange("(t p) r -> p t r", p=128)
                    .bitcast(mmdt),
                )
                for t in range(2):
                    jb = 2 * it + t
                    for c in range(nch):
                        nc.tensor.matmul(
                            axt_ps[:, c * ch : (c + 1) * ch],
                            mm(x_sb[:, jb, :]),
                            mm(buf[:, t, c * ch : (c + 1) * ch]),
                            start=(jb == 0),
                            stop=(jb == nb - 1),
                        )
            for c in range(nch):
                nc.vector.tensor_copy(
                    axt_sb[:, c * ch : (c + 1) * ch], axt_ps[:, c * ch : (c + 1) * ch]
                )

            # ---- z, softmax(s), grouped in PSUM-bank-sized batches ----
            grp = min(4, rb)
            for g in range(rb // grp):
                lo, hi = g * grp, (g + 1) * grp
                z_ps = pmm.tile([128, grp, F], f32, tag="zps")
                sl_ps = pmm.tile([128, grp, F], f32, tag="zps")
                for k in range(grp):
                    ib = lo + k
                    nc.tensor.matmul(
                        z_ps[:, k, :], axt_sb[:, ib * 128 : (ib + 1) * 128], wet_sb[:]
                    )
                    nc.tensor.matmul(
                        sl_ps[:, k, :], axt_sb[:, ib * 128 : (ib + 1) * 128], wat_sb[:]
                    )
                d_bc = d_rc[:, lo:hi].broadcast_to((128, grp, F))
                nc.vector.tensor_tensor(
                    z_sb[:, lo:hi, :], z_ps[:], d_bc, op=mybir.AluOpType.mult
                )
                nc.vector.tensor_tensor(
                    sl_sb[:, lo:hi, :], sl_ps[:], d_bc, op=mybir.AluOpType.mult
                )
            bias_e = beb_sb[:].broadcast_to((128, F, rb)).rearrange("p f b -> p b f")
            bias_a = bab_sb[:].broadcast_to((128, F, rb)).rearrange("p f b -> p b f")
            nc.vector.tensor_tensor(z_sb[:], z_sb[:], bias_e, op=mybir.AluOpType.add)
            nc.vector.tensor_tensor(sl_sb[:], sl_sb[:], bias_a, op=mybir.AluOpType.add)
            nc.vector.reduce_max(
                mx[:], sl_sb[:], axis=mybir.AxisListType.X, negate=True
            )
            for ib in range(rb):
                nc.scalar.activation(
                    sl_sb[:, ib, :],
                    sl_sb[:, ib, :],
                    mybir.ActivationFunctionType.Exp,
                    bias=mx[:, ib, :],
                    accum_out=ssum[:, ib, :],
                )
            nc.vector.reciprocal(rsum[:], ssum[:])
            nc.vector.tensor_tensor(
                sl_sb[:],
                sl_sb[:],
                rsum[:].broadcast_to((128, rb, F)),
                op=mybir.AluOpType.mult,
            )

            # s shard out + AllGather (fp16 payload) + hoisted X_next partial
            xn_ps = pfin.tile([F, F], f32, tag="xnps")
            nc.gpsimd.dma_start(
                s_out.ap().rearrange("(b p) f -> p b f", p=128), sl_sb[:]
            )
            nc.vector.tensor_copy(s_bf[:], sl_sb[:])
            nc.gpsimd.dma_start(
                cc_s_in.ap().rearrange("(b p) f -> p b f", p=128), s_bf[:]
            )
            nc.gpsimd.collective_compute(
                "AllGather",
                mybir.AluOpType.bypass,
                replica_groups=rgroups,
                ins=[cc_s_in.ap()],
                outs=[cc_s_out.ap()],
            )
            for ib in range(rb):
                nc.tensor.matmul(
                    xn_ps[:],
                    sl_sb[:, ib, :],
                    z_sb[:, ib, :],
                    start=(ib == 0),
                    stop=(ib == rb - 1),
                )

            # ---- P3: AhatS_T accumulation over j blocks ----
            # Reverse j order so the first `nres` buffers are P1's residents:
            # P3's matmuls start right after the AllGather with no strip DMA.
            ast_ps = pacc.tile([128, rows], f32, tag="bigacc")
            order = list(reversed(range(nb // 2)))
            for pos, it in enumerate(order):
                if it in res_tiles:
                    buf = res_tiles[it]
                else:
                    buf = p3b.tile([128, 2, rows], f16, tag="bbuf")
                    nc.sync.dma_start(
                        buf[:],
                        atb.ap()[it * 256 : (it + 1) * 256, :].rearrange(
                            "(t p) r -> p t r", p=128
                        ),
                    )
                for t in range(2):
                    jb = 2 * it + t
                    sf = sfp.tile([128, F], f16, tag="sfull")
                    nc.scalar.dma_start(
                        sf[:], cc_s_out.ap()[jb * 128 : (jb + 1) * 128, :]
                    )
                    for c in range(nch):
                        nc.tensor.matmul(
                            ast_ps[:, c * ch : (c + 1) * ch],
                            sf[:],
                            buf[:, t, c * ch : (c + 1) * ch],
                            start=(pos == 0 and t == 0),
                            stop=(pos == nb // 2 - 1 and t == 1),
                        )
            for c in range(nch):
                nc.vector.tensor_copy(
                    ast_sb[:, c * ch : (c + 1) * ch], ast_ps[:, c * ch : (c + 1) * ch]
                )

            # As = AhatS_T.T - s_loc ; then final contraction
            an_ps = pfin.tile([F, F], f32, tag="anps")
            for ib in range(rb):
                tr_ps = pmm.tile([128, 128], f32, tag="zps")
                nc.tensor.transpose(
                    tr_ps[:], ast_sb[:, ib * 128 : (ib + 1) * 128], ident[:]
                )
                nc.vector.tensor_sub(as_sb[:, ib, :], tr_ps[:], sl_sb[:, ib, :])
            for ib in range(rb):
                nc.tensor.matmul(
                    an_ps[:],
                    sl_sb[:, ib, :],
                    as_sb[:, ib, :],
                    start=(ib == 0),
                    stop=(ib == rb - 1),
                )
            nc.vector.tensor_copy(xn_sb[:], xn_ps[:])
            nc.vector.tensor_copy(an_sb[:], an_ps[:])
            nc.gpsimd.dma_start(xn_out.ap(), xn_sb[:])
            nc.gpsimd.dma_start(an_out.ap(), an_sb[:])

    nc.compile()
    return nc


def make_in_maps(X, A, W_embed, b_embed, W_assign, b_assign, n_cores=CORES_FULL):
    """Host-side sharding / layout prep. Returns one input dict per core."""
    X = np.ascontiguousarray(np.asarray(X, dtype=np.float32))
    A = np.asarray(A, dtype=np.float32)
    n = A.shape[0]
    rows = n // n_cores
    we_t = np.ascontiguousarray(np.asarray(W_embed, np.float32).T)
    wa_t = np.ascontiguousarray(np.asarray(W_assign, np.float32).T)
    be_b = np.ascontiguousarray(
        np.broadcast_to(np.asarray(b_embed, np.float32)[None, :], (128, F))
    )
    ba_b = np.ascontiguousarray(
        np.broadcast_to(np.asarray(b_assign, np.float32)[None, :], (128, F))
    )
    in_maps = []
    for c in range(n_cores):
        at = np.ascontiguousarray(A[c * rows : (c + 1) * rows, :].T)
        # bake A_hat = A + I into the shard
        at[np.arange(c * rows, (c + 1) * rows), np.arange(rows)] += 1.0
        in_maps.append(
            {
                "at_hat": at,
                "at_f16": at.astype(np.float16),
                "x_in": X,
                "we_t": we_t,
                "wa_t": wa_t,
                "be_b": be_b,
                "ba_b": ba_b,
            }
        )
    return in_maps


_CACHE = {}


def _get_program(n, n_cores):
    key = (n, n_cores)
    if key not in _CACHE:
        _CACHE[key] = build_program(n, n_cores)
    return _CACHE[key]


def run_on_hw(inputs, n_cores=CORES_FULL, trace=False):
    """Compile (cached), run on hardware, return (outputs_tuple, BassKernelResults)."""
    from concourse.bass_interp import get_hw_module

    n = inputs["A"].shape[0]
    nc = _get_program(n, n_cores)
    in_maps = make_in_maps(n_cores=n_cores, **inputs)
    old_m = nc.m
    nc.m = get_hw_module(nc.m)
    try:
        res = run_bass_kernel_spmd(
            nc, in_maps, core_ids=list(range(n_cores)), trace=trace
        )
    finally:
        nc.m = old_m
    outs = res.results
    x_next = np.sum([r["xn_part"] for r in outs], axis=0, dtype=np.float32)
    a_next = np.sum([r["an_part"] for r in outs], axis=0, dtype=np.float32)
    s_l = np.concatenate([r["s_part"] for r in outs], axis=0)
    return (x_next, a_next, s_l), res


def kernel(X, A, W_embed, b_embed, W_assign, b_assign):
    out, _ = run_on_hw(
        dict(
            X=X,
            A=A,
            W_embed=W_embed,
            b_embed=b_embed,
            W_assign=W_assign,
            b_assign=b_assign,
        )
    )
    return out


# revision 58
# speedup vs baseline: 1.5622x; 1.0446x over previous
"""DiffPool forward (GCN embed/assign + pooled X/A) on 8 trn2 NeuronCores.

Sharding: 1D node partition. Core c owns rows R_c = [c*ROWS, (c+1)*ROWS).
Host feeds each core A_T_hat_c = (A[R_c,:] + I[R_c,:]).T  -> [N, ROWS], so the
contraction index j sits on the SBUF partition axis for both big products and
no transposes of A are ever done on device.

Device program (single SPMD NEFF, collectives across 8 cores):
  P1: deg_loc = column-sums of fp16 A_T_hat strips via ones-vector matmuls on
      the otherwise-idle PE (4-way tile_position col packing). The tail strip
      buffers stay resident in SBUF for P3 to reuse.
      AllGather(deg) -> d = 1/sqrt(deg)   (a tiny warmup collective issued at
      program start absorbs the ~80us ncfw first-collective cost under P1)
  P2: AHXT[f,i]   = sum_j (d*X)[j,f] * A_T_hat[j,i]   (fp32r PSUM accum, f32 strips)
      z   = d_i * (AHXT.T @ We.T) + be
      s   = softmax(d_i * (AHXT.T @ Wa.T) + ba)       (row softmax, free axis)
      AllGather(s in fp16) -> s_full
  P3: AhatS_T[m,i] = sum_j s_full[j,m] * A_T_hat[j,i] (fp16 strips)
      As = AhatS_T.T - s_loc                          (undo baked +I)
      Xn_part = s_loc.T @ z ; An_part = s_loc.T @ As  (partials over local rows)
Host: X_next = sum_c Xn_part, A_next = sum_c An_part, s_l = concat_c(s_part).

Precision: P2 (mixed-sign contraction) keeps fp32-width data (fp32r matmul
mode, ~1e-5 rel err). deg and A@s are nonnegative sums of ~8192 terms, so
independent fp16 input-rounding attenuates by ~1/sqrt(N): fp16 there costs
~1e-5..1e-4 rel err on A_next only, and halves those passes' DMA.
"""

import numpy as np

import concourse.bass as bass
import concourse.mybir as mybir
import concourse.tile as tile
from concourse import bacc
from concourse.bass_utils import run_bass_kernel_spmd
from concourse.masks import make_identity

F = 128  # F_IN == N_META == N_EMBED == 128
f32 = mybir.dt.float32
N_FULL = 8192
CORES_FULL = 8


def build_program(n=N_FULL, n_cores=CORES_FULL, fast32=True):
    """Build the SPMD single-core program (same NEFF on all cores)."""
    rows = n // n_cores      # local node rows per core
    nb = n // 128            # j blocks (global)
    rb = rows // 128         # local row blocks
    ch = min(512, rows)      # matmul moving-dim chunk
    nch = rows // ch         # chunks per strip
    assert nb % 2 == 0 and rows % 128 == 0
    mmdt = mybir.dt.float32r if fast32 else f32

    def mm(ap):
        return ap.bitcast(mmdt)

    nc = bacc.Bacc(
        "TRN2",
        target_bir_lowering=False,
        debug=False,
        enable_asserts=False,
        num_devices=n_cores,
    )

    f16 = mybir.dt.float16
    at = nc.dram_tensor("at_hat", [n, rows], f32, kind="ExternalInput")
    atb = nc.dram_tensor("at_f16", [n, rows], f16, kind="ExternalInput")
    x_in = nc.dram_tensor("x_in", [n, F], f32, kind="ExternalInput")
    wet = nc.dram_tensor("we_t", [F, F], f32, kind="ExternalInput")
    wat = nc.dram_tensor("wa_t", [F, F], f32, kind="ExternalInput")
    beb = nc.dram_tensor("be_b", [128, F], f32, kind="ExternalInput")
    bab = nc.dram_tensor("ba_b", [128, F], f32, kind="ExternalInput")

    s_out = nc.dram_tensor("s_part", [rows, F], f32, kind="ExternalOutput")
    xn_out = nc.dram_tensor("xn_part", [F, F], f32, kind="ExternalOutput")
    an_out = nc.dram_tensor("an_part", [F, F], f32, kind="ExternalOutput")

    cc_warm_in = nc.dram_tensor("cc_warm_in", [8], f32, kind="Internal")
    cc_warm_out = nc.dram_tensor(
        "cc_warm_out", [8 * n_cores], f32, kind="Internal", addr_space="Shared"
    )
    cc_deg_in = nc.dram_tensor("cc_deg_in", [rows], f32, kind="Internal")
    cc_deg_out = nc.dram_tensor(
        "cc_deg_out", [n], f32, kind="Internal", addr_space="Shared"
    )
    cc_s_in = nc.dram_tensor("cc_s_in", [rows, F], f16, kind="Internal")
    cc_s_out = nc.dram_tensor(
        "cc_s_out", [n, F], f16, kind="Internal", addr_space="Shared"
    )
    rgroups = [list(range(n_cores))]

    with tile.TileContext(nc) as tc:
        with (
            tc.tile_pool(name="strips", bufs=13) as strips,
            tc.tile_pool(name="p3b", bufs=6) as p3b,
            tc.tile_pool(name="persist", bufs=1) as persist,
            tc.tile_pool(name="sf", bufs=4) as sfp,
            tc.tile_pool(name="pacc", bufs=1, space="PSUM") as pacc,
            tc.tile_pool(name="pmm", bufs=2, space="PSUM") as pmm,
            tc.tile_pool(name="pfin", bufs=1, space="PSUM") as pfin,
        ):
            # ---- persistent sbuf tensors ----
            x_sb = persist.tile([128, nb, F], f32)      # X, later d*X (j on partitions)
            ones16 = persist.tile([128, 1], f16)
            deg_line = persist.tile([1, rows], f32)
            d_sb = persist.tile([128, nb], f32)         # 1/sqrt(deg) full, [p, blk]
            d_rc = persist.tile([128, rb], f32)         # same for local rows
            wet_sb = persist.tile([F, F], f32)
            wat_sb = persist.tile([F, F], f32)
            beb_sb = persist.tile([128, F], f32)
            bab_sb = persist.tile([128, F], f32)
            ident = persist.tile([128, 128], f32)
            axt_sb = persist.tile([128, rows], f32)     # AHXT
            z_sb = persist.tile([128, rb, F], f32)
            sl_sb = persist.tile([128, rb, F], f32)     # logits -> s
            s_bf = persist.tile([128, rb, F], f16)      # s in fp16 for P3
            as_sb = persist.tile([128, rb, F], f32)     # A @ s (local rows)
            ast_sb = persist.tile([128, rows], f32)     # AhatS_T
            mx = persist.tile([128, rb, 1], f32)
            ssum = persist.tile([128, rb, 1], f32)
            rsum = persist.tile([128, rb, 1], f32)
            xn_sb = persist.tile([F, F], f32)
            an_sb = persist.tile([F, F], f32)

            # ---- constant loads / setup ----
            # nc.sync is reserved for the big strip streams; small loads go on
            # gpsimd/scalar so a dependent DMA never blocks strip prefetch.
            nc.sync.dma_start(
                mm(x_sb[:]), x_in.ap().rearrange("(b p) f -> p b f", p=128).bitcast(mmdt)
            )
            nc.gpsimd.dma_start(wet_sb[:], wet.ap())
            nc.gpsimd.dma_start(wat_sb[:], wat.ap())
            nc.gpsimd.dma_start(beb_sb[:], beb.ap())
            nc.gpsimd.dma_start(bab_sb[:], bab.ap())
            make_identity(nc, ident[:])
            nc.vector.memset(ones16[:], 1.0)

            # warmup collective: absorbs ncfw first-collective startup cost
            # concurrently with P1's DMA stream
            warm_sb = persist.tile([1, 8 * n_cores], f32)
            nc.vector.memset(warm_sb[:], 0.0)
            nc.gpsimd.dma_start(
                cc_warm_in.ap().rearrange("(a b) -> a b", a=1), warm_sb[:, 0:8]
            )
            nc.gpsimd.collective_compute(
                "AllGather",
                mybir.AluOpType.bypass,
                replica_groups=rgroups,
                ins=[cc_warm_in.ap()],
                outs=[cc_warm_out.ap()],
            )
            nc.gpsimd.dma_start(
                warm_sb[:], cc_warm_out.ap().rearrange("(a b) -> a b", a=1)
            )

            # ---- P1: degrees = column sums of the fp16 transposed shard,
            # computed as ones-vector matmuls on the otherwise-idle PE.
            # 4-way col-group packing (tile_position) runs 4 M=1 matmuls
            # concurrently in the array. at_f16 has A_hat baked in, so the
            # +1 self-loop is included.
            deg_ps = pacc.tile([128, rows], f32, tag="bigacc")
            for it in range(nb // 2):
                buf = p3b.tile([128, 2, rows], f16, tag="bbuf")
                nc.sync.dma_start(
                    buf[:],
                    atb.ap()[it * 256 : (it + 1) * 256, :].rearrange(
                        "(t p) r -> p t r", p=128
                    ),
                )
                for t in range(2):
                    jb = 2 * it + t
                    k = jb % 4
                    for c in range(nch):
                        nc.tensor.matmul(
                            deg_ps[32 * k : 32 * k + 1, c * ch : (c + 1) * ch],
                            ones16[:],
                            buf[:, t, c * ch : (c + 1) * ch],
                            start=(jb == k),
                            stop=(jb == nb - 4 + k),
                            tile_position=(0, 32 * k),
                        )
            nc.vector.tensor_copy(deg_line[:], deg_ps[0:1, :])
            for k in range(1, 4):
                nc.vector.tensor_add(
                    deg_line[:], deg_line[:], deg_ps[32 * k : 32 * k + 1, :]
                )

            # AllGather degrees; local d read back from the DRAM bounce so no
            # core-dependent indexing is needed anywhere in the program
            nc.gpsimd.dma_start(
                cc_deg_in.ap().rearrange("(a b) -> a b", a=1), deg_line[:]
            )
            nc.scalar.dma_start(
                d_rc[:], cc_deg_in.ap().rearrange("(b p) -> p b", p=128)
            )
            nc.scalar.sqrt(d_rc[:], d_rc[:])
            nc.vector.reciprocal(d_rc[:], d_rc[:])
            nc.gpsimd.collective_compute(
                "AllGather",
                mybir.AluOpType.bypass,
                replica_groups=rgroups,
                ins=[cc_deg_in.ap()],
                outs=[cc_deg_out.ap()],
            )
            # chunked d readback + rsqrt + X scale, pipelined so P2's first
            # matmuls start as soon as chunk 0 is ready
            dq = max(1, nb // 8)
            for g in range(nb // dq):
                sl = slice(g * dq, (g + 1) * dq)
                nc.scalar.dma_start(
                    d_sb[:, sl],
                    cc_deg_out.ap()[g * dq * 128 : (g + 1) * dq * 128].rearrange(
                        "(b p) -> p b", p=128
                    ),
                )
                nc.scalar.sqrt(d_sb[:, sl], d_sb[:, sl])
                nc.vector.reciprocal(d_sb[:, sl], d_sb[:, sl])
                nc.vector.tensor_tensor(
                    mm(x_sb[:, sl, :]),
                    x_sb[:, sl, :],
                    d_sb[:, sl].broadcast_to((128, dq, F)),
                    op=mybir.AluOpType.mult,
                )

            # ---- P2: AHXT accumulation over j blocks ----
            axt_ps = pacc.tile([128, rows], f32, tag="bigacc")
            for it in range(nb // 2):
                buf = strips.tile([128, 2, rows], f32, tag="abuf")
                nc.sync.dma_start(
                    mm(buf[:]),
                    at.ap()[it * 256 : (it + 1) * 256, :]
                    .rearrange("(t p) r -> p t r", p=128)
                    .bitcast(mmdt),
                )
                for t in range(2):
                    jb = 2 * it + t
                    for c in range(nch):
                        nc.tensor.matmul(
                            axt_ps[:, c * ch : (c + 1) * ch],
                            mm(x_sb[:, jb, :]),
                            mm(buf[:, t, c * ch : (c + 1) * ch]),
                            start=(jb == 0),
                            stop=(jb == nb - 1),
                        )
            for c in range(nch):
                nc.vector.tensor_copy(
                    axt_sb[:, c * ch : (c + 1) * ch], axt_ps[:, c * ch : (c + 1) * ch]
                )

            # ---- z, softmax(s), grouped in PSUM-bank-sized batches ----
            grp = min(4, rb)
            for g in range(rb // grp):
                lo, hi = g * grp, (g + 1) * grp
                z_ps = pmm.tile([128, grp, F], f32, tag="zps")
                sl_ps = pmm.tile([128, grp, F], f32, tag="zps")
                for k in range(grp):
                    ib = lo + k
                    nc.tensor.matmul(
                        z_ps[:, k, :], axt_sb[:, ib * 128 : (ib + 1) * 128], wet_sb[:]
                    )
                    nc.tensor.matmul(
                        sl_ps[:, k, :], axt_sb[:, ib * 128 : (ib + 1) * 128], wat_sb[:]
                    )
                d_bc = d_rc[:, lo:hi].broadcast_to((128, grp, F))
                nc.vector.tensor_tensor(
                    z_sb[:, lo:hi, :], z_ps[:], d_bc, op=mybir.AluOpType.mult
                )
                nc.vector.tensor_tensor(
                    sl_sb[:, lo:hi, :], sl_ps[:], d_bc, op=mybir.AluOpType.mult
                )
            bias_e = beb_sb[:].broadcast_to((128, F, rb)).rearrange("p f b -> p b f")
            bias_a = bab_sb[:].broadcast_to((128, F, rb)).rearrange("p f b -> p b f")
            nc.vector.tensor_tensor(z_sb[:], z_sb[:], bias_e, op=mybir.AluOpType.add)
            nc.vector.tensor_tensor(sl_sb[:], sl_sb[:], bias_a, op=mybir.AluOpType.add)
            nc.vector.reduce_max(
                mx[:], sl_sb[:], axis=mybir.AxisListType.X, negate=True
            )
            for ib in range(rb):
                nc.scalar.activation(
                    sl_sb[:, ib, :],
                    sl_sb[:, ib, :],
                    mybir.ActivationFunctionType.Exp,
                    bias=mx[:, ib, :],
                    accum_out=ssum[:, ib, :],
                )
            nc.vector.reciprocal(rsum[:], ssum[:])
            nc.vector.tensor_tensor(
                sl_sb[:],
                sl_sb[:],
                rsum[:].broadcast_to((128, rb, F)),
                op=mybir.AluOpType.mult,
            )

            # s shard out + AllGather (fp16 payload) + hoisted X_next partial
            xn_ps = pfin.tile([F, F], f32, tag="xnps")
            nc.gpsimd.dma_start(
                s_out.ap().rearrange("(b p) f -> p b f", p=128), sl_sb[:]
            )
            nc.vector.tensor_copy(s_bf[:], sl_sb[:])
            nc.gpsimd.dma_start(
                cc_s_in.ap().rearrange("(b p) f -> p b f", p=128), s_bf[:]
            )
            nc.gpsimd.collective_compute(
                "AllGather",
                mybir.AluOpType.bypass,
                replica_groups=rgroups,
                ins=[cc_s_in.ap()],
                outs=[cc_s_out.ap()],
            )
            for ib in range(rb):
                nc.tensor.matmul(
                    xn_ps[:],
                    sl_sb[:, ib, :],
                    z_sb[:, ib, :],
                    start=(ib == 0),
                    stop=(ib == rb - 1),
                )

            # ---- P3: AhatS_T accumulation over j blocks ----
            ast_ps = pacc.tile([128, rows], f32, tag="bigacc")
            for it in range(nb // 2):
                buf = p3b.tile([128, 2, rows], f16, tag="bbuf")
                nc.sync.dma_start(
                    buf[:],
                    atb.ap()[it * 256 : (it + 1) * 256, :].rearrange(
                        "(t p) r -> p t r", p=128
                    ),
                )
                sfb = sfp.tile([128, 2, F], f16, tag="sfull")
                nc.scalar.dma_start(
                    sfb[:],
                    cc_s_out.ap()[it * 256 : (it + 1) * 256, :].rearrange(
                        "(t p) f -> p t f", p=128
                    ),
                )
                for t in range(2):
                    jb = 2 * it + t
                    for c in range(nch):
                        nc.tensor.matmul(
                            ast_ps[:, c * ch : (c + 1) * ch],
                            sfb[:, t, :],
                            buf[:, t, c * ch : (c + 1) * ch],
                            start=(jb == 0),
                            stop=(jb == nb - 1),
                        )
            for c in range(nch):
                nc.vector.tensor_copy(
                    ast_sb[:, c * ch : (c + 1) * ch], ast_ps[:, c * ch : (c + 1) * ch]
                )

            # As = AhatS_T.T - s_loc ; then final contraction
            an_ps = pfin.tile([F, F], f32, tag="anps")
            for ib in range(rb):
                tr_ps = pmm.tile([128, 128], f32, tag="zps")
                nc.tensor.transpose(
                    tr_ps[:], ast_sb[:, ib * 128 : (ib + 1) * 128], ident[:]
                )
                nc.vector.tensor_sub(as_sb[:, ib, :], tr_ps[:], sl_sb[:, ib, :])
            for ib in range(rb):
                nc.tensor.matmul(
                    an_ps[:],
                    sl_sb[:, ib, :],
                    as_sb[:, ib, :],
                    start=(ib == 0),
                    stop=(ib == rb - 1),
                )
            nc.vector.tensor_copy(xn_sb[:], xn_ps[:])
            nc.vector.tensor_copy(an_sb[:], an_ps[:])
            nc.gpsimd.dma_start(xn_out.ap(), xn_sb[:])
            nc.gpsimd.dma_start(an_out.ap(), an_sb[:])

    nc.compile()
    return nc


def make_in_maps(X, A, W_embed, b_embed, W_assign, b_assign, n_cores=CORES_FULL):
    """Host-side sharding / layout prep. Returns one input dict per core."""
    X = np.ascontiguousarray(np.asarray(X, dtype=np.float32))
    A = np.asarray(A, dtype=np.float32)
    n = A.shape[0]
    rows = n // n_cores
    we_t = np.ascontiguousarray(np.asarray(W_embed, np.float32).T)
    wa_t = np.ascontiguousarray(np.asarray(W_assign, np.float32).T)
    be_b = np.ascontiguousarray(
        np.broadcast_to(np.asarray(b_embed, np.float32)[None, :], (128, F))
    )
    ba_b = np.ascontiguousarray(
        np.broadcast_to(np.asarray(b_assign, np.float32)[None, :], (128, F))
    )
    in_maps = []
    for c in range(n_cores):
        at = np.ascontiguousarray(A[c * rows : (c + 1) * rows, :].T)
        # bake A_hat = A + I into the shard
        at[np.arange(c * rows, (c + 1) * rows), np.arange(rows)] += 1.0
        in_maps.append(
            {
                "at_hat": at,
                "at_f16": at.astype(np.float16),
                "x_in": X,
                "we_t": we_t,
                "wa_t": wa_t,
                "be_b": be_b,
                "ba_b": ba_b,
            }
        )
    return in_maps


_CACHE = {}


def _get_program(n, n_cores):
    key = (n, n_cores)
    if key not in _CACHE:
        _CACHE[key] = build_program(n, n_cores)
    return _CACHE[key]


def run_on_hw(inputs, n_cores=CORES_FULL, trace=False):
    """Compile (cached), run on hardware, return (outputs_tuple, BassKernelResults)."""
    from concourse.bass_interp import get_hw_module

    n = inputs["A"].shape[0]
    nc = _get_program(n, n_cores)
    in_maps = make_in_maps(n_cores=n_cores, **inputs)
    old_m = nc.m
    nc.m = get_hw_module(nc.m)
    try:
        res = run_bass_kernel_spmd(
            nc, in_maps, core_ids=list(range(n_cores)), trace=trace
        )
    finally:
        nc.m = old_m
    outs = res.results
    x_next = np.sum([r["xn_part"] for r in outs], axis=0, dtype=np.float32)
    a_next = np.sum([r["an_part"] for r in outs], axis=0, dtype=np.float32)
    s_l = np.concatenate([r["s_part"] for r in outs], axis=0)
    return (x_next, a_next, s_l), res


def kernel(X, A, W_embed, b_embed, W_assign, b_assign):
    out, _ = run_on_hw(
        dict(
            X=X,
            A=A,
            W_embed=W_embed,
            b_embed=b_embed,
            W_assign=W_assign,
            b_assign=b_assign,
        )
    )
    return out
